# revision 1
# baseline (speedup 1.0000x reference)
"""Cross-attention kernel for Trainium2, 8 NeuronCores.

Reference computation (B=4, S=2048, C=1024, E=1024, D=768, H=16, hd=64):
    q = x @ q_w + q_b                 # [B,S,E]
    k = context @ k_w + k_b           # [B,C,E]
    v = context @ v_w + v_b           # [B,C,E]
    attn = softmax(q.k^T / sqrt(hd))  # per head
    out = (attn @ v) @ o_w + o_b      # [B,S,E]

Sharding: 8 cores = 4 batches x 2 head-groups (8 heads = 512 embed cols each).
Each core computes the full attention for its (batch, head-group) and a
partial out-projection; the host sums the two head-group partials per batch
(the "all-reduce") and adds o_b.

Device layout: everything is computed in a transposed orientation so no
on-device transposes are needed.  The host passes x^T and context^T; the
projections produce Q^T/K^T with the head dim on partitions and V in natural
layout.  Scores are computed transposed (S^T = K @ Q^T, contraction over
hd=64, two heads packed into the 128-row PE array via row groups), the
softmax denominator comes free from the attention@V matmul by appending a
ones column to V (stationary operand is [V_h | 1], M=65), and the final
normalization is a per-column multiply using a gpsimd partition-broadcast of
the reciprocal sums.  All matmuls run as float32r (fp22 multiply, fp32
accumulate) which is full-rate on the PE for 512-wide moving operands.

The attention inner loop is ACT-bound (two exps of [128,512] per c-step vs
three matmul-slots of PE work), so the emission is software-pipelined: the
Q-projection matmuls for s-tile n+1 and the out-projection matmuls for
s-tile n-1 are interleaved into attention(n)'s c-steps to keep the PE fed
while the scalar engine works through the exps.
"""

import sys

sys.path.insert(0, "/opt/trn_rl_repo")

import numpy as np

B, S, E, C, D = 4, 2048, 1024, 1024, 768
H, HD = 16, 64
EL = E // 2          # embed columns per head-group (8 heads)
N_CORES = 8
NS = S // 512        # s-tiles of 512
KE = E // 128        # contraction chunks for q-proj
KD = D // 128        # contraction chunks for k/v-proj
NC2 = C // 512       # c-tiles of 512
CC = C // 128        # c chunks of 128
HP = EL // 128       # head pairs per core (4)

# "fp32r" (fp22 multiply, ~2.8e-4 end-to-end rel err) or "fp16"
# (halves DMA traffic and SBUF, ~1e-3 rel err)
DTYPE_MODE = "fp32r"

_built = None
_last_results = None


def _build(reps=1, nop_us=0, mode=None):
    import concourse.bacc as bacc
    import concourse.mybir as mybir
    from concourse.tile import TileContext

    F32 = mybir.dt.float32
    F32R = mybir.dt.float32r
    F16 = mybir.dt.float16
    Exp = mybir.ActivationFunctionType.Exp
    Ident = mybir.ActivationFunctionType.Identity

    if mode is None:
        mode = DTYPE_MODE
    CT = F32R if mode == "fp32r" else F16   # compute dtype for matmul operands
    IN = F32 if mode == "fp32r" else F16    # dram dtype for matmul inputs

    nc = bacc.Bacc(None, target_bir_lowering=False)

    xT = nc.declare_dram_parameter("xT", [E, S], IN, isOutput=False)
    ctxT = nc.declare_dram_parameter("ctxT", [D, C], IN, isOutput=False)
    qw = nc.declare_dram_parameter("qw", [E, EL], IN, isOutput=False)
    kw = nc.declare_dram_parameter("kw", [D, EL], IN, isOutput=False)
    vw = nc.declare_dram_parameter("vw", [D, EL], IN, isOutput=False)
    ow = nc.declare_dram_parameter("ow", [EL, E], IN, isOutput=False)
    qb = nc.declare_dram_parameter("qb", [EL, 1], F32, isOutput=False)
    kb = nc.declare_dram_parameter("kb", [EL, 1], F32, isOutput=False)
    vb = nc.declare_dram_parameter("vb", [1, EL], IN, isOutput=False)
    ones_r = nc.declare_dram_parameter("ones_r", [1, 128], IN, isOutput=False)
    out = nc.declare_dram_parameter("out", [S, E], F32, isOutput=True)

    def r(ap):
        return ap.bitcast(F32R) if mode == "fp32r" else ap

    with TileContext(nc) as tc:
        with (
            tc.tile_pool(name="wpool", bufs=1) as wpool,
            tc.tile_pool(name="dpool", bufs=1) as dpool,
            tc.tile_pool(name="xpool", bufs=4) as xpool,
            tc.tile_pool(name="qtpool", bufs=8) as qtpool,
            tc.tile_pool(name="ptpool", bufs=4) as ptpool,
            tc.tile_pool(name="otpool", bufs=8) as otpool,
            tc.tile_pool(name="spool", bufs=2) as spool,
            tc.tile_pool(name="opool", bufs=2) as opool,
            tc.tile_pool(name="pspool", bufs=1, space="PSUM") as pspool,
        ):
          for _rep in range(reps):
            # ---- weight / bias / context loads ---------------------------
            # One strided mega-DMA per tensor (chunks packed side-by-side in
            # a single SBUF tile, per-chunk views sliced out): each dma_start
            # holds the global HWDGE issue slot ~625ns, so fewer+bigger wins.
            # Ordered by first use: kw+ctx(first half) -> vw -> rest.
            def chunked_tile(pool, nchunk, width, name):
                t = pool.tile([128, nchunk * width], CT, name=name)
                return t, [t[:, i * width:(i + 1) * width] for i in range(nchunk)]

            # per-chunk DMAs for the prologue-critical tensors so the PE can
            # start as soon as the first chunks land
            _, kw_sb = chunked_tile(wpool, KD, EL, "kw_all")
            _, vw_sb = chunked_tile(wpool, KD, EL, "vw_all")
            ctx_all = dpool.tile([128, KD * C], CT, name="ctx_all")
            ctx_sb = [ctx_all[:, d * C:(d + 1) * C] for d in range(KD)]
            for d in range(KD):
                nc.sync.dma_start(
                    out=kw_sb[d][:], in_=r(kw[d * 128:(d + 1) * 128, :]))
                nc.sync.dma_start(
                    out=ctx_sb[d][:, 0:512],
                    in_=r(ctxT[d * 128:(d + 1) * 128, 0:512]))
            for d in range(KD):
                nc.sync.dma_start(
                    out=vw_sb[d][:], in_=r(vw[d * 128:(d + 1) * 128, :]))
            for d in range(KD):
                nc.sync.dma_start(
                    out=ctx_sb[d][:, 512:1024],
                    in_=r(ctxT[d * 128:(d + 1) * 128, 512:1024]))
            kb_t = wpool.tile([128, HP], F32, name="kb_t")
            nc.sync.dma_start(
                out=kb_t.rearrange("p (c w) -> p c w", w=1),
                in_=kb.rearrange("(c p) w -> p c w", p=128),
            )
            kb_sb = [kb_t[:, m:m + 1] for m in range(HP)]
            qb_t = wpool.tile([128, HP], F32, name="qb_t")
            nc.sync.dma_start(
                out=qb_t.rearrange("p (c w) -> p c w", w=1),
                in_=qb.rearrange("(c p) w -> p c w", p=128),
            )
            qb_sb = [qb_t[:, m:m + 1] for m in range(HP)]
            vb_sb = wpool.tile([1, EL], CT, name="vb_sb")
            nc.sync.dma_start(out=vb_sb[:], in_=r(vb[:]))
            ones_sb = wpool.tile([1, 128], CT, name="ones_sb")
            nc.sync.dma_start(out=ones_sb[:], in_=r(ones_r[:]))
            vb_bc = wpool.tile([128, EL], F32, name="vb_bc")
            vb_ps = pspool.tile([128, 512], F32, name="acc_ps", tag="acc", bufs=2)
            nc.tensor.matmul(vb_ps[:], ones_sb[0:1, :], vb_sb[:],
                             start=True, stop=True)
            nc.vector.tensor_copy(vb_bc[:], vb_ps[:])
            _, qw_sb = chunked_tile(wpool, KE, EL, "qw_all")
            for k in range(KE):
                nc.sync.dma_start(
                    out=qw_sb[k][:], in_=r(qw[k * 128:(k + 1) * 128, :]))
            ow_all = wpool.tile([128, HP * E], CT, name="ow_all")
            ow_sb = [ow_all[:, k * E:(k + 1) * E] for k in range(HP)]

            def load_ow():
                nc.sync.dma_start(
                    out=ow_all.rearrange("p (c w) -> p c w", w=E),
                    in_=r(ow).rearrange("(c p) w -> p c w", p=128),
                )

            # ---- K^T projection: [EL rows, C cols], head pairs on partitions --
            kt_sb = []
            for m in range(HP):
                t = dpool.tile([128, C], CT, name=f"kt{m}")
                kt_sb.append(t)

            def kt_thunks(m, t2s=range(NC2)):
                """Matmul thunks computing K^T halves for head pair m."""
                state = {}
                thunks = []

                def f(t2, d):
                    if d == 0:
                        state[t2] = pspool.tile(
                            [128, 512], F32, name="acc_ps", tag="acc", bufs=2)
                    ps = state[t2]
                    nc.tensor.matmul(
                        ps[:],
                        kw_sb[d][:, m * 128:(m + 1) * 128],
                        ctx_sb[d][:, t2 * 512:(t2 + 1) * 512],
                        start=(d == 0), stop=(d == KD - 1),
                    )
                    if d == KD - 1:
                        nc.vector.tensor_scalar_add(
                            kt_sb[m][:, t2 * 512:(t2 + 1) * 512], ps[:],
                            kb_sb[m][:, 0:1],
                        )

                for t2 in t2s:
                    for d in range(KD):
                        thunks.append((f, t2, d))
                return thunks

            # ---- V projection: natural [C rows, EL cols], interleaved with a
            # ones column per head for the softmax denominator ------------------
            v_sb = []
            for mc in range(CC):
                t = dpool.tile([128, 8 * 65], CT, name=f"v{mc}")
                v_sb.append(t)

            def vproj_group(mc):
                t = v_sb[mc]
                ps = pspool.tile([128, 512], F32, name="acc_ps", tag="acc", bufs=2)
                for d in range(KD):
                    nc.tensor.matmul(
                        ps[:],
                        ctx_sb[d][:, mc * 128:(mc + 1) * 128],
                        vw_sb[d][:],
                        start=(d == 0), stop=(d == KD - 1),
                    )
                vv = t.rearrange("p (h u) -> p h u", u=65)
                nc.vector.tensor_add(
                    vv[:, :, 0:64],
                    ps.rearrange("p (h u) -> p h u", u=64),
                    vb_bc.rearrange("p (h u) -> p h u", u=64),
                )
                nc.vector.tensor_scalar(
                    vv[:, :, 64:65],
                    vb_bc[:, 0:8].rearrange("p (h u) -> p h u", u=1),
                    0.0, 1.0,
                    mybir.AluOpType.mult, mybir.AluOpType.add,
                )  # writes the constant 1.0 column

            # ---- pipelined main loop over s-tiles of 512 ----------------------
            xts_all = {}
            qts_all = {}
            ots_all = {}

            def load_x(n):
                tiles = []
                for half in range(2):
                    t = xpool.tile([128, 4 * 512], CT, name="xt", tag="xt")
                    views = [t[:, i * 512:(i + 1) * 512] for i in range(4)]
                    if n == 0:
                        # n=0 is on the startup critical path: per-chunk DMAs
                        for i in range(4):
                            k = half * 4 + i
                            nc.sync.dma_start(
                                out=views[i][:],
                                in_=r(xT[k * 128:(k + 1) * 128,
                                         n * 512:(n + 1) * 512]))
                    else:
                        nc.sync.dma_start(
                            out=t.rearrange("p (c w) -> p c w", w=512),
                            in_=r(xT[half * 512:(half + 1) * 512,
                                     n * 512:(n + 1) * 512])
                            .rearrange("(c p) w -> p c w", p=128),
                        )
                    tiles += views
                xts_all[n] = tiles

            def qproj_thunks(n):
                """32 matmul thunks computing Q^T for s-tile n (4 psum groups)."""
                state = {}
                thunks = []
                qts_all[n] = [None] * HP

                def f(m, k):
                    if k == 0:
                        state[m] = pspool.tile(
                            [128, 512], F32, name="acc_ps", tag="acc", bufs=2)
                    ps = state[m]
                    nc.tensor.matmul(
                        ps[:],
                        qw_sb[k][:, m * 128:(m + 1) * 128],
                        xts_all[n][k][:],
                        start=(k == 0), stop=(k == KE - 1),
                    )
                    if k == KE - 1:
                        qt_t = qtpool.tile([128, 512], CT, name="qt", tag="qt")
                        nc.vector.tensor_scalar_add(qt_t[:], ps[:], qb_sb[m][:, 0:1])
                        qts_all[n][m] = qt_t

                for m in range(HP):
                    for k in range(KE):
                        thunks.append((f, m, k))
                return thunks

            def outproj_thunks(n):
                """32 matmul thunks for the out-projection of s-tile n."""
                state = {}
                thunks = []

                def f(ss, ne, hp):
                    if hp == 0:
                        state[(ss, ne)] = pspool.tile(
                            [128, 512], F32, name="acc_ps", tag="acc", bufs=2)
                        if ne == 0:
                            state[ss] = opool.tile(
                                [128, 1024], F32, name="o_sb", tag="o")
                    ps = state[(ss, ne)]
                    nc.tensor.matmul(
                        ps[:],
                        ots_all[n][hp][:, ss * 128:(ss + 1) * 128],
                        ow_sb[hp][:, ne * 512:(ne + 1) * 512],
                        start=(hp == 0), stop=(hp == HP - 1),
                    )
                    if hp == HP - 1:
                        o_sb = state[ss]
                        nc.vector.tensor_copy(
                            o_sb[:, ne * 512:(ne + 1) * 512], ps[:])
                        if ne == 1:
                            nc.sync.dma_start(
                                out=out[n * 512 + ss * 128:
                                        n * 512 + (ss + 1) * 128, :],
                                in_=o_sb[:],
                            )

                for ss in range(4):
                    for ne in range(2):
                        for hp in range(HP):
                            thunks.append((f, ss, ne, hp))
                return thunks

            def run_thunks(ts):
                for f, *args in ts:
                    f(*args)

            # prologue, ordered to match DMA arrival (kw+ctx.h1, vw, ctx.h2,
            # qw+xT0): K^T m=0 and V directly, then Q^T(0) m=0; the other head
            # pairs' K^T and Q^T groups ride in attention(0)'s background,
            # ordered so each lands before the head pair that needs it.
            load_x(0)
            load_ow()
            run_thunks(kt_thunks(0, t2s=[0]))
            for mc in range(4):
                vproj_group(mc)
            run_thunks(kt_thunks(0, t2s=[1]))
            for mc in range(4, CC):
                vproj_group(mc)
            qp0 = qproj_thunks(0)
            run_thunks(qp0[:KE])          # m=0 group
            prologue_bg = []
            for m in range(1, HP):
                prologue_bg += kt_thunks(m)
                prologue_bg += qp0[m * KE:(m + 1) * KE]

            for n in range(NS):
                if n + 1 < NS:
                    load_x(n + 1)
                bg = []
                if n == 0:
                    bg += prologue_bg
                if n + 1 < NS:
                    bg += qproj_thunks(n + 1)
                if n >= 1:
                    bg += outproj_thunks(n - 1)

                ots_all[n] = [None] * HP
                qts = qts_all[n]
                n_steps = HP * CC
                step = 0
                bg_done = 0
                for hp in range(HP):
                    ovs = [
                        pspool.tile([65, 512], F32, name="ov_ps", tag="ov", bufs=2)
                        for _ in range(2)
                    ]
                    for c in range(CC):
                        pts = []
                        for h2 in range(2):
                            sc = pspool.tile(
                                [128, 512], F32, name="sc_ps", tag="sc", bufs=3)
                            # scores^T block: K_h @ Q_h^T, contraction hd=64.
                            # h2=0 uses PE rows 0-63, h2=1 rows 64-127 -> the
                            # two matmuls run concurrently in row groups.
                            nc.tensor.matmul(
                                sc[:],
                                kt_sb[hp][h2 * 64:(h2 + 1) * 64,
                                          c * 128:(c + 1) * 128],
                                qts[hp][h2 * 64:(h2 + 1) * 64, :],
                                start=True, stop=True,
                            )
                            p = ptpool.tile([128, 512], CT, name="pt", tag="pt")
                            nc.scalar.activation(p[:], sc[:], Exp)
                            pts.append(p)
                        # inject background (q-proj n+1 / out-proj n-1) work
                        # between the scores and the exp-gated AV matmuls so
                        # the PE stays busy through the exp latency
                        step += 1
                        target = step * len(bg) // n_steps
                        while bg_done < target:
                            f, *args = bg[bg_done]
                            f(*args)
                            bg_done += 1
                        for h2 in range(2):
                            h = hp * 2 + h2
                            nc.tensor.matmul(
                                ovs[h2][:],
                                v_sb[c][:, h * 65:(h + 1) * 65],
                                pts[h2][:],
                                start=(c == 0), stop=(c == CC - 1),
                            )
                    # normalization epilogue for this head pair
                    ot_t = otpool.tile([128, 512], CT, name="ot", tag="ot")
                    for h2 in range(2):
                        rs = spool.tile([1, 512], CT, name="rs", tag="rs")
                        with nc.allow_low_precision("softmax denom, fp22 ok"):
                            nc.vector.reciprocal(rs[:], ovs[h2][64:65, :])
                        bc_ps = pspool.tile([64, 512], F32, name="bc_ps",
                                            tag="bc", bufs=1)
                        nc.tensor.matmul(bc_ps[:], ones_sb[0:1, 0:64], rs[:],
                                         start=True, stop=True)
                        bc = spool.tile([64, 512], F32, name="bc", tag="bc")
                        nc.vector.tensor_copy(bc[:], bc_ps[:])
                        nc.vector.tensor_mul(
                            ot_t[h2 * 64:(h2 + 1) * 64, :], ovs[h2][0:64, :], bc[:]
                        )
                    ots_all[n][hp] = ot_t
                run_thunks(bg[bg_done:])

            # epilogue: out-projection of the last s-tile
            run_thunks(outproj_thunks(NS - 1))

          # timing aid: calibrated delay chain on the otherwise-idle gpsimd
          # engine; kernel exec time = max(real work, nop chain)
          if nop_us:
            NOP_CYC = 48000  # 40 us at 1.2 GHz
            for _ in range(int(nop_us * 1200 / NOP_CYC)):
                nc.gpsimd.nop(cycle_cnt=NOP_CYC, nofuse=True)

    nc.finalize()
    return nc


def kernel(x, context, q_w, q_b, k_w, k_b, v_w, v_b, o_w, o_b):
    global _built, _last_results
    from concourse.bass_utils import run_bass_kernel_spmd

    if _built is None:
        _built = _build()
    nc = _built

    scale = np.float32(1.0 / np.sqrt(HD))
    ind = np.float32 if DTYPE_MODE == "fp32r" else np.float16
    x = np.asarray(x, np.float32)
    context = np.asarray(context, np.float32)
    xTs = [np.ascontiguousarray(x[b].T).astype(ind) for b in range(B)]
    ctxTs = [np.ascontiguousarray(context[b].T).astype(ind) for b in range(B)]

    in_maps = []
    for core in range(N_CORES):
        b, hg = core // 2, core % 2
        el = slice(hg * EL, (hg + 1) * EL)
        in_maps.append({
            "xT": xTs[b],
            "ctxT": ctxTs[b],
            "qw": np.ascontiguousarray(
                (np.asarray(q_w, np.float32)[:, el] * scale).astype(ind)),
            "kw": np.ascontiguousarray(np.asarray(k_w, np.float32)[:, el]).astype(ind),
            "vw": np.ascontiguousarray(np.asarray(v_w, np.float32)[:, el]).astype(ind),
            "ow": np.ascontiguousarray(np.asarray(o_w, np.float32)[el, :]).astype(ind),
            "qb": np.ascontiguousarray(
                (np.asarray(q_b, np.float32)[el] * scale)[:, None]),
            "kb": np.ascontiguousarray(np.asarray(k_b, np.float32)[el][:, None]),
            "vb": np.ascontiguousarray(
                np.asarray(v_b, np.float32)[el][None, :]).astype(ind),
            "ones_r": np.ones((1, 128), ind),
        })

    res = run_bass_kernel_spmd(nc, in_maps, list(range(N_CORES)))
    _last_results = res

    ob = np.asarray(o_b, np.float32)
    full = np.empty((B, S, E), np.float32)
    for b in range(B):
        full[b] = res.results[2 * b]["out"] + res.results[2 * b + 1]["out"] + ob
    return full



# revision 26
# speedup vs baseline: 1.1952x; 1.1952x over previous
"""Cross-attention kernel for Trainium2, 8 NeuronCores.

Reference computation (B=4, S=2048, C=1024, E=1024, D=768, H=16, hd=64):
    q = x @ q_w + q_b                 # [B,S,E]
    k = context @ k_w + k_b           # [B,C,E]
    v = context @ v_w + v_b           # [B,C,E]
    attn = softmax(q.k^T / sqrt(hd))  # per head
    out = (attn @ v) @ o_w + o_b      # [B,S,E]

Sharding: 8 cores = 4 batches x 2 head-groups (8 heads = 512 embed cols each).
Each core computes the full attention for its (batch, head-group) and a
partial out-projection; the host sums the two head-group partials per batch
(the "all-reduce") and adds o_b.

Device layout: everything is computed in a transposed orientation so the only
on-device transpose is a cheap [128,128] PE transpose per attention block.
The host passes x^T and context^T; the projections produce Q^T/K^T with the
head dim on partitions and V in natural layout.  Scores are computed
transposed (S^T = K @ Q^T, contraction over hd=64, two heads packed into the
128-row PE array via row groups).

The attention@V matmul runs in the cheap orientation: stationary = a
[c=128, s=128] block of P^T (the exp output), moving = V [c=128, 65] (64 head
dims + a ones column that yields the softmax denominator), output
[s=128, 66]-ish per head accumulating over c.  This charges only 65 PE rows
per instruction instead of 512 (the cost model charges by moving-free size),
halving attention@V cost versus the [d, s] orientation.  The normalization is
then a per-partition scalar multiply (reciprocal of the denominator column),
and a PE transpose (fp16 identity, 128 rows) restores the [d, s] layout the
out-projection needs.  All operands are fp16 (full-rate on the PE at any
moving width; fp32r is 4x penalized below 256-wide moving operands).

The attention inner loop is ACT-bound (two exps of [128,512] per c-step vs
~1 matmul-slot of PE work), so the emission is software-pipelined: the
attention@V + normalize + transpose chain for head-pair hp-1, the
Q-projection matmuls for s-tile n+1 and the out-projection matmuls for s-tile
n-1 are interleaved into hp's score/exp steps to keep the PE fed while the
scalar engine works through the exps.
"""

import sys

sys.path.insert(0, "/opt/trn_rl_repo")

import numpy as np

B, S, E, C, D = 4, 2048, 1024, 1024, 768
H, HD = 16, 64
EL = E // 2          # embed columns per head-group (8 heads)
N_CORES = 8
NS = S // 512        # s-tiles of 512
KE = E // 128        # contraction chunks for q-proj
KD = D // 128        # contraction chunks for k/v-proj
NC2 = C // 512       # c-tiles of 512
CC = C // 128        # c chunks of 128
HP = EL // 128       # head pairs per core (4)

# "fp32r" (fp22 multiply) or "fp16" (halves DMA traffic and SBUF, and is
# full-rate on the PE for narrow moving operands, which fp32r is not)
DTYPE_MODE = "fp16"

_built = None
_last_results = None


def _build(reps=1, nop_us=0, mode=None):
    import concourse.bacc as bacc
    import concourse.mybir as mybir
    from concourse.tile import TileContext

    F32 = mybir.dt.float32
    F32R = mybir.dt.float32r
    F16 = mybir.dt.float16
    Exp = mybir.ActivationFunctionType.Exp

    if mode is None:
        mode = DTYPE_MODE
    CT = F32R if mode == "fp32r" else F16   # compute dtype for matmul operands
    IN = F32 if mode == "fp32r" else F16    # dram dtype for matmul inputs

    nc = bacc.Bacc(None, target_bir_lowering=False)

    xT = nc.declare_dram_parameter("xT", [E, S], IN, isOutput=False)
    ctxT = nc.declare_dram_parameter("ctxT", [D, C], IN, isOutput=False)
    qw = nc.declare_dram_parameter("qw", [E, EL], IN, isOutput=False)
    kw = nc.declare_dram_parameter("kw", [D, EL], IN, isOutput=False)
    vw = nc.declare_dram_parameter("vw", [D, EL], IN, isOutput=False)
    ow = nc.declare_dram_parameter("ow", [EL, E], IN, isOutput=False)
    qb = nc.declare_dram_parameter("qb", [EL, 1], F32, isOutput=False)
    kb = nc.declare_dram_parameter("kb", [EL, 1], F32, isOutput=False)
    vb = nc.declare_dram_parameter("vb", [1, EL], IN, isOutput=False)
    ones_r = nc.declare_dram_parameter("ones_r", [1, 128], IN, isOutput=False)
    ident = nc.declare_dram_parameter("ident", [128, 128], F16, isOutput=False)
    out = nc.declare_dram_parameter("out", [S, E], F16, isOutput=True)

    def r(ap):
        return ap.bitcast(F32R) if mode == "fp32r" else ap

    with TileContext(nc) as tc:
        with (
            tc.tile_pool(name="wpool", bufs=1) as wpool,
            tc.tile_pool(name="dpool", bufs=1) as dpool,
            tc.tile_pool(name="xpool", bufs=4) as xpool,
            tc.tile_pool(name="qtpool", bufs=8) as qtpool,
            tc.tile_pool(name="ptpool", bufs=16) as ptpool,
            tc.tile_pool(name="ntpool", bufs=6) as ntpool,
            tc.tile_pool(name="otpool", bufs=8) as otpool,
            tc.tile_pool(name="spool", bufs=4) as spool,
            tc.tile_pool(name="opool", bufs=2) as opool,
            tc.tile_pool(name="pspool", bufs=1, space="PSUM") as pspool,
        ):
          for _rep in range(reps):
            # ---- weight / bias / context loads ---------------------------
            # One strided mega-DMA per tensor (chunks packed side-by-side in
            # a single SBUF tile, per-chunk views sliced out): each dma_start
            # holds the global HWDGE issue slot ~625ns, so fewer+bigger wins
            # (per-chunk DMAs serialize on the issue path and delay the whole
            # prologue far more than the coarser dependency costs).
            # Ordered by first use: kw+ctx(first half)+kb -> vw+ctx2+vb -> qw
            # -> x0 -> rest.
            def chunked_tile(pool, nchunk, width, name):
                t = pool.tile([128, nchunk * width], CT, name=name)
                return t, [t[:, i * width:(i + 1) * width] for i in range(nchunk)]

            def load_mega(t, src, nchunk, width):
                nc.sync.dma_start(
                    out=t.rearrange("p (c w) -> p c w", w=width),
                    in_=src.rearrange("(c p) w -> p c w", p=128),
                )

            # DMA issue order = serial transfer order; ordered by deadline:
            # kt(t2=0) needs kw+ctx1+kb; q-proj(0) needs qw+qb+x0; then the
            # second context half / V-projection / out-proj weights.
            kw_t, kw_sb = chunked_tile(wpool, KD, EL, "kw_all")
            vw_t, vw_sb = chunked_tile(wpool, KD, EL, "vw_all")
            ctx_all = dpool.tile([128, KD * C], CT, name="ctx_all")
            ctx_sb = [ctx_all[:, d * C:(d + 1) * C] for d in range(KD)]
            ctx_3d = ctx_all.rearrange("p (c w) -> p c w", w=C)
            load_mega(kw_t, r(kw), KD, EL)
            nc.sync.dma_start(
                out=ctx_3d[:, :, 0:512],
                in_=r(ctxT)[:, 0:512].rearrange("(c p) w -> p c w", p=128),
            )
            kb_t = wpool.tile([128, HP], F32, name="kb_t")
            nc.sync.dma_start(
                out=kb_t.rearrange("p (c w) -> p c w", w=1),
                in_=kb.rearrange("(c p) w -> p c w", p=128),
            )
            kb_sb = [kb_t[:, m:m + 1] for m in range(HP)]
            qw_t, qw_sb = chunked_tile(wpool, KE, EL, "qw_all")
            load_mega(qw_t, r(qw), KE, EL)
            qb_t = wpool.tile([128, HP], F32, name="qb_t")
            nc.sync.dma_start(
                out=qb_t.rearrange("p (c w) -> p c w", w=1),
                in_=qb.rearrange("(c p) w -> p c w", p=128),
            )
            qb_sb = [qb_t[:, m:m + 1] for m in range(HP)]

            def load_late_weights():
                load_mega(vw_t, r(vw), KD, EL)
                nc.sync.dma_start(
                    out=ctx_3d[:, :, 512:1024],
                    in_=r(ctxT)[:, 512:1024]
                    .rearrange("(c p) w -> p c w", p=128),
                )
                nc.sync.dma_start(out=vb_sb[:], in_=r(vb[:]))
                nc.sync.dma_start(out=ones_sb[:], in_=r(ones_r[:]))

            vb_sb = wpool.tile([1, EL], CT, name="vb_sb")
            ones_sb = wpool.tile([1, 128], CT, name="ones_sb")
            vb_bc = wpool.tile([128, EL], F32, name="vb_bc")

            def vb_chain():
                vb_ps = pspool.tile([128, 512], F32, name="acc_ps",
                                    tag="acc", bufs=2)
                nc.tensor.matmul(vb_ps[:], ones_sb[0:1, :], vb_sb[:],
                                 start=True, stop=True)
                nc.vector.tensor_copy(vb_bc[:], vb_ps[:])

            ident_sb = wpool.tile([128, 128], F16, name="ident_sb")
            ow_all = wpool.tile([128, HP * E], CT, name="ow_all")
            ow_sb = [ow_all[:, k * E:(k + 1) * E] for k in range(HP)]

            def load_ow():
                nc.sync.dma_start(out=ident_sb[:], in_=ident[:])
                nc.sync.dma_start(
                    out=ow_all.rearrange("p (c w) -> p c w", w=E),
                    in_=r(ow).rearrange("(c p) w -> p c w", p=128),
                )

            # ---- K^T projection: [EL rows, C cols], head pairs on partitions --
            kt_sb = []
            for m in range(HP):
                t = dpool.tile([128, C], CT, name=f"kt{m}")
                kt_sb.append(t)

            def kt_thunks(m, t2s=range(NC2)):
                """Matmul thunks computing K^T halves for head pair m."""
                state = {}
                thunks = []

                def f(t2, d):
                    if d == 0:
                        state[t2] = pspool.tile(
                            [128, 512], F32, name="acc_ps", tag="acc", bufs=2)
                    ps = state[t2]
                    nc.tensor.matmul(
                        ps[:],
                        kw_sb[d][:, m * 128:(m + 1) * 128],
                        ctx_sb[d][:, t2 * 512:(t2 + 1) * 512],
                        start=(d == 0), stop=(d == KD - 1),
                    )
                    if d == KD - 1:
                        nc.vector.tensor_scalar_add(
                            kt_sb[m][:, t2 * 512:(t2 + 1) * 512], ps[:],
                            kb_sb[m][:, 0:1],
                        )

                for t2 in t2s:
                    for d in range(KD):
                        thunks.append((f, t2, d))
                return thunks

            # ---- V projection: natural [C rows, EL cols], interleaved with a
            # ones column per head for the softmax denominator ------------------
            v_sb = []
            for mc in range(CC):
                t = dpool.tile([128, 8 * 65], CT, name=f"v{mc}")
                v_sb.append(t)

            def vproj_group(mc):
                t = v_sb[mc]
                ps = pspool.tile([128, 512], F32, name="acc_ps", tag="acc", bufs=2)
                for d in range(KD):
                    nc.tensor.matmul(
                        ps[:],
                        ctx_sb[d][:, mc * 128:(mc + 1) * 128],
                        vw_sb[d][:],
                        start=(d == 0), stop=(d == KD - 1),
                    )
                vv = t.rearrange("p (h u) -> p h u", u=65)
                nc.vector.tensor_add(
                    vv[:, :, 0:64],
                    ps.rearrange("p (h u) -> p h u", u=64),
                    vb_bc.rearrange("p (h u) -> p h u", u=64),
                )
                nc.vector.tensor_scalar(
                    vv[:, :, 64:65],
                    vb_bc[:, 0:8].rearrange("p (h u) -> p h u", u=1),
                    0.0, 1.0,
                    mybir.AluOpType.mult, mybir.AluOpType.add,
                )  # writes the constant 1.0 column

            # ---- pipelined main loop over s-tiles of 512 ----------------------
            xts_all = {}
            qts_all = {}
            ots_all = {}
            pts_all = {}

            def load_x(n):
                tiles = []
                for half in range(2):
                    t = xpool.tile([128, 4 * 512], CT, name="xt", tag="xt")
                    views = [t[:, i * 512:(i + 1) * 512] for i in range(4)]
                    nc.sync.dma_start(
                        out=t.rearrange("p (c w) -> p c w", w=512),
                        in_=r(xT[half * 512:(half + 1) * 512,
                                 n * 512:(n + 1) * 512])
                        .rearrange("(c p) w -> p c w", p=128),
                    )
                    tiles += views
                xts_all[n] = tiles

            def qproj_thunks(n):
                """32 matmul thunks computing Q^T for s-tile n (4 psum groups)."""
                state = {}
                thunks = []
                qts_all[n] = [None] * HP

                def f(m, k):
                    if k == 0:
                        state[m] = pspool.tile(
                            [128, 512], F32, name="acc_ps", tag="acc", bufs=2)
                    ps = state[m]
                    nc.tensor.matmul(
                        ps[:],
                        qw_sb[k][:, m * 128:(m + 1) * 128],
                        xts_all[n][k][:],
                        start=(k == 0), stop=(k == KE - 1),
                    )
                    if k == KE - 1:
                        qt_t = qtpool.tile([128, 512], CT, name="qt", tag="qt")
                        nc.vector.tensor_scalar_add(qt_t[:], ps[:], qb_sb[m][:, 0:1])
                        qts_all[n][m] = qt_t

                for m in range(HP):
                    for k in range(KE):
                        thunks.append((f, m, k))
                return thunks

            def outproj_thunks(n):
                """32 matmul thunks for the out-projection of s-tile n."""
                state = {}
                thunks = []

                def f(ss, ne, hp):
                    if hp == 0:
                        state[(ss, ne)] = pspool.tile(
                            [128, 512], F32, name="acc_ps", tag="acc", bufs=2)
                        if ne == 0:
                            state[ss] = opool.tile(
                                [128, 1024], F16, name="o_sb", tag="o")
                    ps = state[(ss, ne)]
                    nc.tensor.matmul(
                        ps[:],
                        ots_all[n][hp][:, ss * 128:(ss + 1) * 128],
                        ow_sb[hp][:, ne * 512:(ne + 1) * 512],
                        start=(hp == 0), stop=(hp == HP - 1),
                    )
                    if hp == HP - 1:
                        o_sb = state[ss]
                        nc.vector.tensor_copy(
                            o_sb[:, ne * 512:(ne + 1) * 512], ps[:])
                        if ne == 1:
                            nc.sync.dma_start(
                                out=out[n * 512 + ss * 128:
                                        n * 512 + (ss + 1) * 128, :],
                                in_=o_sb[:],
                            )

                for ss in range(4):
                    for ne in range(2):
                        for hp in range(HP):
                            thunks.append((f, ss, ne, hp))
                return thunks

            def av_thunks(n, hp):
                """attention@V + normalize + transpose chain for (n, hp).

                Per s-block sb: 16 matmuls accumulate [s=128, 65]x2 heads into
                one [128,130] psum (col 64 / 129 = softmax denominators from
                the ones column of V), then reciprocal + 2 per-partition
                scalar multiplies normalize into an SBUF tile, and a PE
                transpose (fp16 identity) yields the [d, s] block the
                out-projection consumes.

                Returns (thunks, carry): the normalize/transpose chain lags
                the matmuls by one s-block, and the last block's chain is
                returned as `carry` to be drained at the start of the next
                head pair's steps — the PE transpose sits in the PE stream
                and would otherwise stall it on the DVE norm latency.
                """
                state = {}
                pts = pts_all[(n, hp)]

                def mm(sb, h2, c):
                    # two s-blocks' [128,130] accumulators packed per psum
                    # bank (regions at col 0 and 256)
                    if c == 0 and h2 == 0 and sb % 2 == 0:
                        state[sb // 2] = pspool.tile(
                            [128, 512], F32, name="ov_ps", tag="ov", bufs=2)
                    ps = state[sb // 2]
                    base = (sb % 2) * 256
                    h = hp * 2 + h2
                    nc.tensor.matmul(
                        ps[:, base + h2 * 65:base + (h2 + 1) * 65],
                        pts[(c // 2, h2)][:, (c % 2) * 512
                                          + sb * 128:(c % 2) * 512
                                          + (sb + 1) * 128],
                        v_sb[c][:, h * 65:(h + 1) * 65],
                        start=(c == 0), stop=(c == CC - 1),
                    )

                def norm(sb):
                    ps = state[sb // 2]
                    base = (sb % 2) * 256
                    rs = spool.tile([128, 2], F32, name="rs", tag="rs")
                    with nc.allow_low_precision("softmax denom"):
                        nc.vector.reciprocal(
                            rs.rearrange("p (g u) -> p g u", u=1),
                            ps[:, base:base + 130]
                            .rearrange("p (g u) -> p g u", u=65)[:, :, 64:65])
                    nt = ntpool.tile([128, 128], F16, name="nt", tag="nt")
                    nc.vector.tensor_scalar_mul(
                        nt[:, 0:64], ps[:, base:base + 64], rs[:, 0:1])
                    nc.vector.tensor_scalar_mul(
                        nt[:, 64:128], ps[:, base + 65:base + 129], rs[:, 1:2])
                    state[(sb, "nt")] = nt

                def transp(sb):
                    # transpose output parks in the unused columns of the
                    # already-allocated ov pair tile (as an fp16 view) — no
                    # extra psum bank, no allocation to wait on
                    nt = state[(sb, "nt")]
                    ps = state[sb // 2]
                    base_tr = (sb % 2) * 256 + 136
                    tr = ps[:, base_tr:base_tr + 64].bitcast(F16)
                    nc.tensor.transpose(tr, nt[:], ident_sb[:])
                    state[(sb, "tr")] = tr

                def trcopy(sb):
                    if sb == 0:
                        ots_all[n][hp] = otpool.tile(
                            [128, 512], CT, name="ot", tag="ot")
                    tr = state[(sb, "tr")]
                    nc.vector.tensor_copy(
                        ots_all[n][hp][:, sb * 128:(sb + 1) * 128], tr)

                def mms(sb):
                    return [(mm, sb, h2, c) for h2 in range(2) for c in range(CC)]

                def ntc(sb):
                    return [(norm, sb), (transp, sb), (trcopy, sb)]

                thunks = (mms(0) + mms(1) + ntc(0) + mms(2) + ntc(1)
                          + mms(3) + ntc(2))
                return thunks, ntc(3)

            def run_thunks(ts):
                for f, *args in ts:
                    f(*args)

            # prologue, ordered to match serial DMA arrival (kw+ctx1+kb, qw,
            # x0, vw+ctx2, ow): K^T m=0 first half inline, Q^T(0) m=0 inline;
            # everything else (K^T second half + other head pairs, the whole
            # V projection, remaining Q^T(0) groups) rides in attention(0)'s
            # background, phased by deadline: each head pair's K^T/Q^T lands
            # before the head pair that needs it, V before attention@V(0,0).
            load_x(0)
            load_late_weights()
            load_ow()
            run_thunks(kt_thunks(0, t2s=[0]))
            qp0 = qproj_thunks(0)
            run_thunks(qp0[:KE])          # m=0 group
            prologue_phases = [
                kt_thunks(0, t2s=[1]) + kt_thunks(1) + qp0[KE:2 * KE],
                ([(vb_chain,)] + [(vproj_group, mc) for mc in range(CC)]
                 + kt_thunks(2) + qp0[2 * KE:3 * KE]),
                kt_thunks(3) + qp0[3 * KE:4 * KE],
                [],
            ]

            carry = []
            for n in range(NS):
                if n + 1 < NS:
                    load_x(n + 1)
                bg = []
                if n + 1 < NS:
                    bg += qproj_thunks(n + 1)
                if n >= 1:
                    bg += outproj_thunks(n - 1)

                ots_all[n] = [None] * HP
                qts = qts_all[n]
                n_steps = HP * CC
                step = 0
                bg_done = 0
                for hp in range(HP):
                    # attention@V chain for the previous head pair rides in
                    # this head pair's score/exp steps
                    if hp > 0:
                        a, newcarry = av_thunks(n, hp - 1)
                    elif n > 0:
                        a, newcarry = av_thunks(n - 1, HP - 1)
                    else:
                        a, newcarry = [], []
                    abg = carry + a
                    carry = newcarry
                    abg_done = 0
                    ph = prologue_phases[hp] if n == 0 else []
                    ph_done = 0
                    pts = {}
                    pts_all[(n, hp)] = pts
                    for cp in range(CC // 2):
                      for h2 in range(2):
                        # scores^T for a c-chunk PAIR into one 2-bank psum
                        # tile so a single exp covers 1024 columns (the
                        # per-instruction ACT access overhead is ~30% at 512).
                        # K_h @ Q_h^T, contraction hd=64; h2=0 uses PE rows
                        # 0-63, h2=1 rows 64-127 (row groups).
                        sc = pspool.tile(
                            [128, 1024], F32, name="sc_ps", tag="sc", bufs=2)
                        for ci in range(2):
                            c = 2 * cp + ci
                            nc.tensor.matmul(
                                sc[:, ci * 512:(ci + 1) * 512],
                                kt_sb[hp][h2 * 64:(h2 + 1) * 64,
                                          c * 128:(c + 1) * 128],
                                qts[hp][h2 * 64:(h2 + 1) * 64, :],
                                start=True, stop=True,
                            )
                        p = ptpool.tile([128, 1024], CT, name="pt", tag="pt")
                        nc.scalar.activation(p[:], sc[:], Exp)
                        pts[(cp, h2)] = p
                        # inject background work (attention@V chain for hp-1,
                        # q-proj n+1 / out-proj n-1) between the score steps so
                        # the PE stays busy through the exp latency
                        step += 1
                        stepin = cp * 2 + h2 + 1
                        # startup work (s-tile 0) phased by deadline
                        ptarget = min(len(ph), stepin * len(ph) // (CC - 1))
                        while ph_done < ptarget:
                            f, *args = ph[ph_done]
                            f(*args)
                            ph_done += 1
                        # attention@V chain; lagged on (0,1) while the V
                        # projection is still landing
                        lag = 3 if (n == 0 and hp == 1) else 0
                        if stepin <= lag:
                            atarget = 0
                        else:
                            atarget = min(len(abg), (stepin - lag) * len(abg)
                                          // (CC - 1 - lag))
                        while abg_done < atarget:
                            f, *args = abg[abg_done]
                            f(*args)
                            abg_done += 1
                        target = step * len(bg) // n_steps
                        while bg_done < target:
                            f, *args = bg[bg_done]
                            f(*args)
                            bg_done += 1
                    run_thunks(ph[ph_done:])
                    run_thunks(abg[abg_done:])
                run_thunks(bg[bg_done:])

            # epilogue: attention@V for the last head pair interleaved with
            # the out-projection of the last s-tile (each out-proj group's
            # hp<3 matmuls only need earlier head pairs' ot tiles)
            a, newcarry = av_thunks(NS - 1, HP - 1)
            av_all = carry + a + newcarry
            oth = outproj_thunks(NS - 1)
            # av_all layout: [carry][mms0 16][mms1 16][ntc0 3][mms2 16][ntc1 3]
            # [mms3 16][ntc2 3][ntc3 3]; out-proj group ss needs ntc(ss) done.
            nca = len(carry)
            cuts = [nca + 35, nca + 54, nca + 73, len(av_all)]
            merged = []
            ai = 0
            for ss in range(4):
                merged += av_all[ai:cuts[ss]]
                merged += oth[ss * 8:(ss + 1) * 8]
                ai = cuts[ss]
            run_thunks(merged)

          # timing aid: calibrated delay chain on the otherwise-idle gpsimd
          # engine; kernel exec time = max(real work, nop chain)
          if nop_us:
            NOP_CYC = 48000  # 40 us at 1.2 GHz
            for _ in range(int(nop_us * 1200 / NOP_CYC)):
                nc.gpsimd.nop(cycle_cnt=NOP_CYC, nofuse=True)

    nc.finalize()
    return nc


def kernel(x, context, q_w, q_b, k_w, k_b, v_w, v_b, o_w, o_b):
    global _built, _last_results
    from concourse.bass_utils import run_bass_kernel_spmd

    if _built is None:
        _built = _build()
    nc = _built

    scale = np.float32(1.0 / np.sqrt(HD))
    ind = np.float32 if DTYPE_MODE == "fp32r" else np.float16
    x = np.asarray(x, np.float32)
    context = np.asarray(context, np.float32)
    xTs = [np.ascontiguousarray(x[b].T).astype(ind) for b in range(B)]
    ctxTs = [np.ascontiguousarray(context[b].T).astype(ind) for b in range(B)]

    in_maps = []
    for core in range(N_CORES):
        b, hg = core // 2, core % 2
        el = slice(hg * EL, (hg + 1) * EL)
        in_maps.append({
            "xT": xTs[b],
            "ctxT": ctxTs[b],
            "qw": np.ascontiguousarray(
                (np.asarray(q_w, np.float32)[:, el] * scale).astype(ind)),
            "kw": np.ascontiguousarray(np.asarray(k_w, np.float32)[:, el]).astype(ind),
            "vw": np.ascontiguousarray(np.asarray(v_w, np.float32)[:, el]).astype(ind),
            "ow": np.ascontiguousarray(np.asarray(o_w, np.float32)[el, :]).astype(ind),
            "qb": np.ascontiguousarray(
                (np.asarray(q_b, np.float32)[el] * scale)[:, None]),
            "kb": np.ascontiguousarray(np.asarray(k_b, np.float32)[el][:, None]),
            "vb": np.ascontiguousarray(
                np.asarray(v_b, np.float32)[el][None, :]).astype(ind),
            "ones_r": np.ones((1, 128), ind),
            "ident": np.eye(128, dtype=np.float16),
        })

    res = run_bass_kernel_spmd(nc, in_maps, list(range(N_CORES)))
    _last_results = res

    ob = np.asarray(o_b, np.float32)
    full = np.empty((B, S, E), np.float32)
    for b in range(B):
        full[b] = (res.results[2 * b]["out"].astype(np.float32)
                   + res.results[2 * b + 1]["out"].astype(np.float32) + ob)
    return full


# revision 45
# speedup vs baseline: 1.2385x; 1.0362x over previous
"""Cross-attention kernel for Trainium2, 8 NeuronCores.

Reference computation (B=4, S=2048, C=1024, E=1024, D=768, H=16, hd=64):
    q = x @ q_w + q_b                 # [B,S,E]
    k = context @ k_w + k_b           # [B,C,E]
    v = context @ v_w + v_b           # [B,C,E]
    attn = softmax(q.k^T / sqrt(hd))  # per head
    out = (attn @ v) @ o_w + o_b      # [B,S,E]

Sharding: 8 cores = 4 batches x 2 head-groups (8 heads = 512 embed cols each).
Each core computes the full attention for its (batch, head-group) and a
partial out-projection; the host sums the two head-group partials per batch
(the "all-reduce") and adds o_b.

Device layout: everything is computed in a transposed orientation so the only
on-device transpose is a cheap [128,128] PE transpose per attention block.
The host passes x^T and context^T; the projections produce Q^T/K^T with the
head dim on partitions and V in natural layout.  Scores are computed
transposed (S^T = K @ Q^T, contraction over hd=64, two heads packed into the
128-row PE array via row groups), a c-chunk PAIR at a time into one 2-bank
[128,1024] psum tile so a single exp covers 1024 columns (amortizes the
~185ns per-instruction ACT access overhead).

The attention@V matmul runs in the cheap orientation: stationary = a
[c=128, s=128] block of P^T (the exp output), moving = V [c=128, 65] (64 head
dims + a ones column that yields the softmax denominator), output [s=128,
130] for a head pair accumulating over c.  This charges only 65 PE rows per
instruction instead of 512 (the cost model charges by moving-free size),
halving attention@V cost versus the [d, s] orientation.  The normalization is
then a per-partition scalar multiply (reciprocal of the denominator columns),
and a PE transpose (fp16 identity, 128 rows, output parked in unused columns
of the ov psum tile) restores the [d, s] layout the out-projection needs.
All matmul operands are fp16 (full-rate on the PE at any moving width; fp32r
is 4x penalized below 256-wide moving operands); the output partials are
stored fp16 (halves the serial out-DMA traffic) and summed fp32 on the host.

Scheduling: DMA transfers serialize on one pipe in transfer-issue order, so
the prologue issues deadline-ordered mega-DMAs (kw+ctx1+kb, qw+qb, x0, then
ctx2+vw) and defers compute-gated DMAs so they never block the queue.  The
emission is software-pipelined: the attention@V + normalize + transpose
chain for the head pair TWO windows back (catching up to one window in
s-tile 2), the Q-projection for s-tile n+1 (steps 1-16), the out-projection
for s-tile n-1 (steps 13-32), and on s-tile 0 the phased K^T/V/Q^T prologue
projections are all interleaved into each head pair's score/exp steps to
keep the PE fed while the scalar engine works through the exps.
"""

import sys

sys.path.insert(0, "/opt/trn_rl_repo")

import numpy as np

B, S, E, C, D = 4, 2048, 1024, 1024, 768
H, HD = 16, 64
EL = E // 2          # embed columns per head-group (8 heads)
N_CORES = 8
NS = S // 512        # s-tiles of 512
KE = E // 128        # contraction chunks for q-proj
KD = D // 128        # contraction chunks for k/v-proj
NC2 = C // 512       # c-tiles of 512
CC = C // 128        # c chunks of 128
HP = EL // 128       # head pairs per core (4)

# "fp32r" (fp22 multiply) or "fp16" (halves DMA traffic and SBUF, and is
# full-rate on the PE for narrow moving operands, which fp32r is not)
DTYPE_MODE = "fp16"
# scores matmul in fp8e4m3 DoubleRow mode (2x PE throughput on the scores).
# Disabled: with the pipelined schedule the kernel is ACT/latency-bound, so
# fp8 scores only bought ~1.3% while costing 30x the accuracy margin (and the
# fp8 path produced NaNs on the interpreter run).
SCORES_FP8 = False

_built = None
_last_results = None


def _build(reps=1, nop_us=0, mode=None):
    import concourse.bacc as bacc
    import concourse.mybir as mybir
    from concourse.tile import TileContext

    F32 = mybir.dt.float32
    F32R = mybir.dt.float32r
    F16 = mybir.dt.float16
    F8 = mybir.dt.float8e4
    DR = mybir.MatmulPerfMode.DoubleRow
    Exp = mybir.ActivationFunctionType.Exp

    if mode is None:
        mode = DTYPE_MODE
    CT = F32R if mode == "fp32r" else F16   # compute dtype for matmul operands
    IN = F32 if mode == "fp32r" else F16    # dram dtype for matmul inputs

    nc = bacc.Bacc(None, target_bir_lowering=False)

    xT = nc.declare_dram_parameter("xT", [E, S], IN, isOutput=False)
    ctxT = nc.declare_dram_parameter("ctxT", [D, C], IN, isOutput=False)
    qw = nc.declare_dram_parameter("qw", [E, EL], IN, isOutput=False)
    kw = nc.declare_dram_parameter("kw", [D, EL], IN, isOutput=False)
    vw = nc.declare_dram_parameter("vw", [D, EL], IN, isOutput=False)
    ow = nc.declare_dram_parameter("ow", [EL, E], IN, isOutput=False)
    qb = nc.declare_dram_parameter("qb", [EL, 1], F32, isOutput=False)
    kb = nc.declare_dram_parameter("kb", [EL, 1], F32, isOutput=False)
    vb = nc.declare_dram_parameter("vb", [1, EL], IN, isOutput=False)
    ones_r = nc.declare_dram_parameter("ones_r", [1, 128], IN, isOutput=False)
    ident = nc.declare_dram_parameter("ident", [128, 128], F16, isOutput=False)
    out = nc.declare_dram_parameter("out", [S, E], F16, isOutput=True)

    def r(ap):
        return ap.bitcast(F32R) if mode == "fp32r" else ap

    with TileContext(nc) as tc:
        with (
            tc.tile_pool(name="wpool", bufs=1) as wpool,
            tc.tile_pool(name="dpool", bufs=1) as dpool,
            tc.tile_pool(name="xpool", bufs=4) as xpool,
            tc.tile_pool(name="qtpool", bufs=8) as qtpool,
            tc.tile_pool(name="ptpool", bufs=32) as ptpool,
            tc.tile_pool(name="ntpool", bufs=6) as ntpool,
            tc.tile_pool(name="otpool", bufs=8) as otpool,
            tc.tile_pool(name="spool", bufs=4) as spool,
            tc.tile_pool(name="opool", bufs=2) as opool,
            tc.tile_pool(name="pspool", bufs=1, space="PSUM") as pspool,
        ):
          for _rep in range(reps):
            # ---- weight / bias / context loads ---------------------------
            # One strided mega-DMA per tensor (chunks packed side-by-side in
            # a single SBUF tile, per-chunk views sliced out): each dma_start
            # holds the global HWDGE issue slot ~625ns, so fewer+bigger wins
            # (per-chunk DMAs serialize on the issue path and delay the whole
            # prologue far more than the coarser dependency costs).
            # Ordered by first use: kw+ctx(first half)+kb -> vw+ctx2+vb -> qw
            # -> x0 -> rest.
            def chunked_tile(pool, nchunk, width, name):
                t = pool.tile([128, nchunk * width], CT, name=name)
                return t, [t[:, i * width:(i + 1) * width] for i in range(nchunk)]

            def load_mega(t, src, nchunk, width):
                nc.sync.dma_start(
                    out=t.rearrange("p (c w) -> p c w", w=width),
                    in_=src.rearrange("(c p) w -> p c w", p=128),
                )

            # DMA issue order = serial transfer order; ordered by deadline:
            # kt(t2=0) needs kw+ctx1+kb; q-proj(0) needs qw+qb+x0; then the
            # second context half / V-projection / out-proj weights.
            kw_t, kw_sb = chunked_tile(wpool, KD, EL, "kw_all")
            vw_t, vw_sb = chunked_tile(wpool, KD, EL, "vw_all")
            ctx_all = dpool.tile([128, KD * C], CT, name="ctx_all")
            ctx_sb = [ctx_all[:, d * C:(d + 1) * C] for d in range(KD)]
            ctx_3d = ctx_all.rearrange("p (c w) -> p c w", w=C)
            load_mega(kw_t, r(kw), KD, EL)
            nc.sync.dma_start(
                out=ctx_3d[:, :, 0:512],
                in_=r(ctxT)[:, 0:512].rearrange("(c p) w -> p c w", p=128),
            )
            kb_t = wpool.tile([128, HP], F32, name="kb_t")
            nc.sync.dma_start(
                out=kb_t.rearrange("p (c w) -> p c w", w=1),
                in_=kb.rearrange("(c p) w -> p c w", p=128),
            )
            kb_sb = [kb_t[:, m:m + 1] for m in range(HP)]
            # act-table prewarm: a dummy exp as soon as kb lands loads the
            # Exp LUT (1.28us) while the PE still waits on weight DMAs
            warm_t = wpool.tile([128, 4], F32, name="warm_t")
            nc.scalar.activation(warm_t[:], kb_t[:], Exp)
            qw_t, qw_sb = chunked_tile(wpool, KE, EL, "qw_all")
            load_mega(qw_t, r(qw), KE, EL)
            qb_t = wpool.tile([128, HP], F32, name="qb_t")
            nc.sync.dma_start(
                out=qb_t.rearrange("p (c w) -> p c w", w=1),
                in_=qb.rearrange("(c p) w -> p c w", p=128),
            )
            qb_sb = [qb_t[:, m:m + 1] for m in range(HP)]

            def load_late_weights():
                nc.sync.dma_start(
                    out=ctx_3d[:, :, 512:1024],
                    in_=r(ctxT)[:, 512:1024]
                    .rearrange("(c p) w -> p c w", p=128),
                )
                load_mega(vw_t, r(vw), KD, EL)
                nc.sync.dma_start(out=vb_sb[:], in_=r(vb[:]))
                nc.sync.dma_start(out=ones_sb[:], in_=r(ones_r[:]))

            vb_sb = wpool.tile([1, EL], CT, name="vb_sb")
            ones_sb = wpool.tile([1, 128], CT, name="ones_sb")
            vb_bc = wpool.tile([128, EL], F32, name="vb_bc")

            def vb_chain():
                vb_ps = pspool.tile([128, 512], F32, name="acc_ps",
                                    tag="acc", bufs=2)
                nc.tensor.matmul(vb_ps[:], ones_sb[0:1, :], vb_sb[:],
                                 start=True, stop=True)
                nc.vector.tensor_copy(vb_bc[:], vb_ps[:])

            ident_sb = wpool.tile([128, 128], F16, name="ident_sb")
            ow_all = wpool.tile([128, HP * E], CT, name="ow_all")
            ow_sb = [ow_all[:, k * E:(k + 1) * E] for k in range(HP)]

            def load_ow():
                nc.sync.dma_start(out=ident_sb[:], in_=ident[:])
                nc.sync.dma_start(
                    out=ow_all.rearrange("p (c w) -> p c w", w=E),
                    in_=r(ow).rearrange("(c p) w -> p c w", p=128),
                )

            # ---- K^T projection: [EL rows, C cols], head pairs on partitions --
            # With SCORES_FP8 the epilogue writes fp8e4m3 and a small SBUF
            # DMA relayouts each head's [64, C] half into [32, 2, C] (the
            # hd-dim split across the free dim) for the DoubleRow matmul.
            ST = F8 if SCORES_FP8 else CT
            kt_sb = []
            kt8v = []
            for m in range(HP):
                t = dpool.tile([128, C], ST, name=f"kt{m}")
                kt_sb.append(t)
                if SCORES_FP8:
                    kt8v.append([
                        dpool.tile([32, 2 * C], F8, name=f"kt8v{m}_{h2}")
                        .rearrange("p (g c) -> p g c", c=C)
                        for h2 in range(2)])

            def kt_relayout(m, t2):
                for h2 in range(2):
                    nc.sync.dma_start(
                        out=kt8v[m][h2][:, :, t2 * 512:(t2 + 1) * 512],
                        in_=kt_sb[m][h2 * 64:(h2 + 1) * 64,
                                     t2 * 512:(t2 + 1) * 512]
                        .rearrange("(g p) c -> p g c", p=32),
                    )

            def kt_thunks(m, t2s=range(NC2)):
                """Matmul thunks computing K^T halves for head pair m."""
                state = {}
                thunks = []

                def f(t2, d):
                    if d == 0:
                        state[t2] = pspool.tile(
                            [128, 512], F32, name="acc_ps", tag="acc", bufs=2)
                    ps = state[t2]
                    nc.tensor.matmul(
                        ps[:],
                        kw_sb[d][:, m * 128:(m + 1) * 128],
                        ctx_sb[d][:, t2 * 512:(t2 + 1) * 512],
                        start=(d == 0), stop=(d == KD - 1),
                    )
                    if d == KD - 1:
                        with nc.allow_low_precision("fp8 scores"):
                            nc.vector.tensor_scalar_add(
                                kt_sb[m][:, t2 * 512:(t2 + 1) * 512], ps[:],
                                kb_sb[m][:, 0:1],
                            )
                        if SCORES_FP8:
                            if defer_dma is not None:
                                defer_dma.append((kt_relayout, m, t2))
                            else:
                                kt_relayout(m, t2)

                for t2 in t2s:
                    for d in range(KD):
                        thunks.append((f, t2, d))
                return thunks

            # ---- V projection: natural [C rows, EL cols], interleaved with a
            # ones column per head for the softmax denominator ------------------
            v_sb = []
            for mc in range(CC):
                t = dpool.tile([128, 8 * 65], CT, name=f"v{mc}")
                v_sb.append(t)

            def vproj_group(mc):
                t = v_sb[mc]
                ps = pspool.tile([128, 512], F32, name="acc_ps", tag="acc", bufs=2)
                for d in range(KD):
                    nc.tensor.matmul(
                        ps[:],
                        ctx_sb[d][:, mc * 128:(mc + 1) * 128],
                        vw_sb[d][:],
                        start=(d == 0), stop=(d == KD - 1),
                    )
                vv = t.rearrange("p (h u) -> p h u", u=65)
                nc.vector.tensor_add(
                    vv[:, :, 0:64],
                    ps.rearrange("p (h u) -> p h u", u=64),
                    vb_bc.rearrange("p (h u) -> p h u", u=64),
                )
                nc.vector.tensor_scalar(
                    vv[:, :, 64:65],
                    vb_bc[:, 0:8].rearrange("p (h u) -> p h u", u=1),
                    0.0, 1.0,
                    mybir.AluOpType.mult, mybir.AluOpType.add,
                )  # writes the constant 1.0 column

            # ---- pipelined main loop over s-tiles of 512 ----------------------
            xts_all = {}
            qts_all = {}
            ots_all = {}
            pts_all = {}

            def load_x(n):
                tiles = []
                for half in range(2):
                    t = xpool.tile([128, 4 * 512], CT, name="xt", tag="xt")
                    views = [t[:, i * 512:(i + 1) * 512] for i in range(4)]
                    nc.sync.dma_start(
                        out=t.rearrange("p (c w) -> p c w", w=512),
                        in_=r(xT[half * 512:(half + 1) * 512,
                                 n * 512:(n + 1) * 512])
                        .rearrange("(c p) w -> p c w", p=128),
                    )
                    tiles += views
                xts_all[n] = tiles

            def qt_relayout(n, m, qt_t):
                v8 = []
                for h2 in range(2):
                    t8 = qtpool.tile([32, 1024], F8, name="qt8v",
                                     tag="qt8v", bufs=16)
                    nc.sync.dma_start(
                        out=t8.rearrange("p (g s) -> p g s", s=512),
                        in_=qt_t[h2 * 64:(h2 + 1) * 64, :]
                        .rearrange("(g p) s -> p g s", p=32),
                    )
                    v8.append(t8.rearrange("p (g s) -> p g s", s=512))
                qts_all[n][m] = v8

            def qproj_thunks(n):
                """32 matmul thunks computing Q^T for s-tile n (4 psum groups)."""
                state = {}
                thunks = []
                qts_all[n] = [None] * HP

                def f(m, k):
                    if k == 0:
                        state[m] = pspool.tile(
                            [128, 512], F32, name="acc_ps", tag="acc", bufs=2)
                    ps = state[m]
                    nc.tensor.matmul(
                        ps[:],
                        qw_sb[k][:, m * 128:(m + 1) * 128],
                        xts_all[n][k][:],
                        start=(k == 0), stop=(k == KE - 1),
                    )
                    if k == KE - 1:
                        qt_t = qtpool.tile([128, 512], ST, name="qt", tag="qt")
                        with nc.allow_low_precision("fp8 scores"):
                            nc.vector.tensor_scalar_add(
                                qt_t[:], ps[:], qb_sb[m][:, 0:1])
                        if SCORES_FP8:
                            if defer_dma is not None:
                                defer_dma.append((qt_relayout, n, m, qt_t))
                            else:
                                qt_relayout(n, m, qt_t)
                        else:
                            qts_all[n][m] = qt_t

                for m in range(HP):
                    for k in range(KE):
                        thunks.append((f, m, k))
                return thunks

            def outproj_thunks(n):
                """32 matmul thunks for the out-projection of s-tile n."""
                state = {}
                thunks = []

                def f(ss, ne, hp):
                    if hp == 0:
                        state[(ss, ne)] = pspool.tile(
                            [128, 512], F32, name="acc_ps", tag="acc", bufs=2)
                        if ne == 0:
                            state[ss] = opool.tile(
                                [128, 1024], F16, name="o_sb", tag="o")
                    ps = state[(ss, ne)]
                    nc.tensor.matmul(
                        ps[:],
                        ots_all[n][hp][:, ss * 128:(ss + 1) * 128],
                        ow_sb[hp][:, ne * 512:(ne + 1) * 512],
                        start=(hp == 0), stop=(hp == HP - 1),
                    )
                    if hp == HP - 1:
                        o_sb = state[ss]
                        nc.vector.tensor_copy(
                            o_sb[:, ne * 512:(ne + 1) * 512], ps[:])
                        if ne == 1:
                            nc.sync.dma_start(
                                out=out[n * 512 + ss * 128:
                                        n * 512 + (ss + 1) * 128, :],
                                in_=o_sb[:],
                            )

                for ss in range(4):
                    for ne in range(2):
                        for hp in range(HP):
                            thunks.append((f, ss, ne, hp))
                return thunks

            def make_av(n, hp):
                """builders for the attention@V + normalize + transpose
                chain for (n, hp).

                Per s-block sb: 16 matmuls accumulate [s=128, 65]x2 heads into
                one [128,130] psum (col 64 / 129 = softmax denominators from
                the ones column of V), then reciprocal + 2 per-partition
                scalar multiplies normalize into an SBUF tile, and a PE
                transpose (fp16 identity) yields the [d, s] block the
                out-projection consumes.

                Returns (thunks, carry): the normalize/transpose chain lags
                the matmuls by one s-block, and the last block's chain is
                returned as `carry` to be drained at the start of the next
                head pair's steps — the PE transpose sits in the PE stream
                and would otherwise stall it on the DVE norm latency.
                """
                state = {}

                def mm(sb, h2, c):
                    pts = pts_all[(n, hp)]
                    # two s-blocks' [128,130] accumulators packed per psum
                    # bank (regions at col 0 and 256)
                    if c == 0 and h2 == 0 and sb % 2 == 0:
                        state[sb // 2] = pspool.tile(
                            [128, 512], F32, name="ov_ps", tag="ov", bufs=2)
                    ps = state[sb // 2]
                    base = (sb % 2) * 256
                    h = hp * 2 + h2
                    nc.tensor.matmul(
                        ps[:, base + h2 * 65:base + (h2 + 1) * 65],
                        pts[(c // 2, h2)][:, (c % 2) * 512
                                          + sb * 128:(c % 2) * 512
                                          + (sb + 1) * 128],
                        v_sb[c][:, h * 65:(h + 1) * 65],
                        start=(c == 0), stop=(c == CC - 1),
                    )

                def norm(sb):
                    ps = state[sb // 2]
                    base = (sb % 2) * 256
                    rs = spool.tile([128, 2], F32, name="rs", tag="rs")
                    with nc.allow_low_precision("softmax denom"):
                        nc.vector.reciprocal(
                            rs.rearrange("p (g u) -> p g u", u=1),
                            ps[:, base:base + 130]
                            .rearrange("p (g u) -> p g u", u=65)[:, :, 64:65])
                    nt = ntpool.tile([128, 128], F16, name="nt", tag="nt")
                    nc.vector.tensor_scalar_mul(
                        nt[:, 0:64], ps[:, base:base + 64], rs[:, 0:1])
                    nc.vector.tensor_scalar_mul(
                        nt[:, 64:128], ps[:, base + 65:base + 129], rs[:, 1:2])
                    state[(sb, "nt")] = nt

                def transp(sb):
                    # transpose output parks in the unused columns of the
                    # already-allocated ov pair tile (as an fp16 view) — no
                    # extra psum bank, no allocation to wait on
                    nt = state[(sb, "nt")]
                    ps = state[sb // 2]
                    base_tr = (sb % 2) * 256 + 136
                    tr = ps[:, base_tr:base_tr + 64].bitcast(F16)
                    nc.tensor.transpose(tr, nt[:], ident_sb[:])
                    state[(sb, "tr")] = tr

                def trcopy(sb):
                    if sb == 0:
                        ots_all[n][hp] = otpool.tile(
                            [128, 512], CT, name="ot", tag="ot")
                    tr = state[(sb, "tr")]
                    nc.vector.tensor_copy(
                        ots_all[n][hp][:, sb * 128:(sb + 1) * 128], tr)

                def ntc(sb):
                    return [(norm, sb), (transp, sb), (trcopy, sb)]

                return mm, ntc

            def av_thunks(n, hp):
                mm, ntc = make_av(n, hp)

                def mms(sb):
                    return [(mm, sb, h2, c) for h2 in range(2) for c in range(CC)]

                thunks = (mms(0) + mms(1) + ntc(0) + mms(2) + ntc(1)
                          + mms(3) + ntc(2))
                return thunks, ntc(3)

            def run_thunks(ts):
                for f, *args in ts:
                    f(*args)

            # prologue, ordered to match serial DMA arrival (kw+ctx1+kb, qw,
            # x0, vw+ctx2, ow): K^T m=0 first half inline, Q^T(0) m=0 inline;
            # everything else (K^T second half + other head pairs, the whole
            # V projection, remaining Q^T(0) groups) rides in attention(0)'s
            # background, phased by deadline: each head pair's K^T/Q^T lands
            # before the head pair that needs it, V before attention@V(0,0).
            load_x(0)
            defer_dma = []
            run_thunks(kt_thunks(0, t2s=[0]))
            run_thunks(kt_thunks(1, t2s=[0]))   # keeps the PE warm until x0
            qp0 = qproj_thunks(0)
            run_thunks(qp0[:KE])          # m=0 group
            load_late_weights()
            # the deferred fp8 relayouts issue AFTER ctx2/vw so their
            # compute-gated sem waits don't block the late weights in the
            # serial DMA queue
            for fdma, *fargs in defer_dma:
                fdma(*fargs)
            defer_dma = None
            prologue_phases = [
                # kt(0,t2=1) must fully drain before step 5 emits the first
                # c>=4 score matmul (reads emitted before their producer are
                # invisible to the dependency tracker)
                (kt_thunks(0, t2s=[1]) + qp0[KE:2 * KE]
                 + kt_thunks(1, t2s=[1])),
                ([(load_ow,), (vb_chain,)]
                 + [(vproj_group, mc) for mc in range(CC)]
                 + kt_thunks(2) + qp0[2 * KE:3 * KE]),
                kt_thunks(3) + qp0[3 * KE:4 * KE],
                [],
            ]

            carry = []
            for n in range(NS):
                if n + 1 < NS:
                    load_x(n + 1)
                bg1 = qproj_thunks(n + 1) if n + 1 < NS else []
                bg2 = outproj_thunks(n - 1) if n >= 1 else []

                ots_all[n] = [None] * HP
                qts = qts_all[n]
                n_steps = HP * CC
                step = 0
                bg1_done = 0
                bg2_done = 0
                for hp in range(HP):
                    # attention@V drains lag their head pair by TWO windows
                    # through s-tiles 0/1 (s-tile 0 is PE-oversubscribed by
                    # the projection prologue), catch up to a one-window lag
                    # in s-tile 2's slack, so the epilogue owes only one av
                    i = n * HP + hp
                    if i < 2 * HP:
                        js = [i - 2] if i >= 2 else []
                    elif i == 2 * HP:
                        js = [i - 2, i - 1]
                    else:
                        js = [i - 1]
                    a, newcarry = [], []
                    for j in js:
                        aj, cj = av_thunks(j // HP, j % HP)
                        a += aj
                        newcarry += cj
                    abg = carry + a
                    carry = newcarry
                    abg_done = 0
                    ph = prologue_phases[hp] if n == 0 else []
                    ph_done = 0
                    pts = {}
                    pts_all[(n, hp)] = pts
                    for cp in range(CC // 2):
                      for h2 in range(2):
                        # scores^T for a c-chunk PAIR into one 2-bank psum
                        # tile so a single exp covers 1024 columns (the
                        # per-instruction ACT access overhead is ~30% at 512).
                        # K_h @ Q_h^T, contraction hd=64; h2=0 uses PE rows
                        # 0-63, h2=1 rows 64-127 (row groups).
                        sc = pspool.tile(
                            [128, 1024], F32, name="sc_ps", tag="sc", bufs=2)
                        for ci in range(2):
                            c = 2 * cp + ci
                            if SCORES_FP8:
                                for sh in range(2):
                                    nc.tensor.matmul(
                                        sc[:, ci * 512 + sh * 256:
                                           ci * 512 + (sh + 1) * 256],
                                        kt8v[hp][h2][:, :,
                                                     c * 128:(c + 1) * 128],
                                        qts[hp][h2][:, :,
                                                    sh * 256:(sh + 1) * 256],
                                        start=True, stop=True,
                                        perf_mode=DR,
                                    )
                            else:
                                nc.tensor.matmul(
                                    sc[:, ci * 512:(ci + 1) * 512],
                                    kt_sb[hp][h2 * 64:(h2 + 1) * 64,
                                              c * 128:(c + 1) * 128],
                                    qts[hp][h2 * 64:(h2 + 1) * 64, :],
                                    start=True, stop=True,
                                )
                        p = ptpool.tile([128, 1024], CT, name="pt", tag="pt")
                        nc.scalar.activation(p[:], sc[:], Exp)
                        pts[(cp, h2)] = p
                        # inject background work (attention@V chain for hp-1,
                        # q-proj n+1 / out-proj n-1) between the score steps so
                        # the PE stays busy through the exp latency
                        step += 1
                        stepin = cp * 2 + h2 + 1
                        # startup work (s-tile 0) phased by deadline
                        ptarget = min(len(ph), stepin * len(ph) // (CC - 1))
                        while ph_done < ptarget:
                            f, *args = ph[ph_done]
                            f(*args)
                            ph_done += 1
                        atarget = min(len(abg),
                                      stepin * len(abg) // (CC - 1))
                        while abg_done < atarget:
                            f, *args = abg[abg_done]
                            f(*args)
                            abg_done += 1
                        # q-proj(n+1) over steps 1-16; out-proj(n-1) over
                        # steps 13-32 (its hp=2/3 stationaries are produced
                        # by the lagged av drains in this s-tile's first half)
                        t1 = min(len(bg1), step * len(bg1) // 16)
                        while bg1_done < t1:
                            f, *args = bg1[bg1_done]
                            f(*args)
                            bg1_done += 1
                        t2 = (0 if step <= 12 else
                              min(len(bg2),
                                  (step - 12) * len(bg2) // (n_steps - 12)))
                        while bg2_done < t2:
                            f, *args = bg2[bg2_done]
                            f(*args)
                            bg2_done += 1
                    run_thunks(ph[ph_done:])
                    run_thunks(abg[abg_done:])
                run_thunks(bg1[bg1_done:])
                run_thunks(bg2[bg2_done:])

            # epilogue: attention@V for the last head pair interleaved with
            # the out-projection of the last s-tile.
            # av layout: [carry][mms0 16][mms1 16][ntc0 3][mms2 16][ntc1 3]
            # [mms3 16][ntc2 3][ntc3 3]; out-proj group ss needs ntc(ss).
            a, newcarry = av_thunks(NS - 1, HP - 1)
            av_all = carry + a + newcarry
            oth = outproj_thunks(NS - 1)
            nca = len(carry)
            cuts = [nca + 35, nca + 54, nca + 73, len(av_all)]
            merged = []
            ai = 0
            for ss in range(4):
                merged += av_all[ai:cuts[ss]]
                merged += oth[ss * 8:(ss + 1) * 8]
                ai = cuts[ss]
            run_thunks(merged)

          # timing aid: calibrated delay chain on the otherwise-idle gpsimd
          # engine; kernel exec time = max(real work, nop chain)
          if nop_us:
            NOP_CYC = 48000  # 40 us at 1.2 GHz
            for _ in range(int(nop_us * 1200 / NOP_CYC)):
                nc.gpsimd.nop(cycle_cnt=NOP_CYC, nofuse=True)

    nc.finalize()
    return nc


def kernel(x, context, q_w, q_b, k_w, k_b, v_w, v_b, o_w, o_b):
    global _built, _last_results
    from concourse.bass_utils import run_bass_kernel_spmd

    if _built is None:
        _built = _build()
    nc = _built

    scale = np.float32(1.0 / np.sqrt(HD))
    ind = np.float32 if DTYPE_MODE == "fp32r" else np.float16
    x = np.asarray(x, np.float32)
    context = np.asarray(context, np.float32)
    xTs = [np.ascontiguousarray(x[b].T).astype(ind) for b in range(B)]
    ctxTs = [np.ascontiguousarray(context[b].T).astype(ind) for b in range(B)]

    in_maps = []
    for core in range(N_CORES):
        b, hg = core // 2, core % 2
        el = slice(hg * EL, (hg + 1) * EL)
        in_maps.append({
            "xT": xTs[b],
            "ctxT": ctxTs[b],
            "qw": np.ascontiguousarray(
                (np.asarray(q_w, np.float32)[:, el] * scale).astype(ind)),
            "kw": np.ascontiguousarray(np.asarray(k_w, np.float32)[:, el]).astype(ind),
            "vw": np.ascontiguousarray(np.asarray(v_w, np.float32)[:, el]).astype(ind),
            "ow": np.ascontiguousarray(np.asarray(o_w, np.float32)[el, :]).astype(ind),
            "qb": np.ascontiguousarray(
                (np.asarray(q_b, np.float32)[el] * scale)[:, None]),
            "kb": np.ascontiguousarray(np.asarray(k_b, np.float32)[el][:, None]),
            "vb": np.ascontiguousarray(
                np.asarray(v_b, np.float32)[el][None, :]).astype(ind),
            "ones_r": np.ones((1, 128), ind),
            "ident": np.eye(128, dtype=np.float16),
        })

    res = run_bass_kernel_spmd(nc, in_maps, list(range(N_CORES)))
    _last_results = res

    ob = np.asarray(o_b, np.float32)
    full = np.empty((B, S, E), np.float32)
    for b in range(B):
        full[b] = (res.results[2 * b]["out"].astype(np.float32)
                   + res.results[2 * b + 1]["out"].astype(np.float32) + ob)
    return full


# revision 53
# speedup vs baseline: 1.2418x; 1.0026x over previous
"""Cross-attention kernel for Trainium2, 8 NeuronCores.

Reference computation (B=4, S=2048, C=1024, E=1024, D=768, H=16, hd=64):
    q = x @ q_w + q_b                 # [B,S,E]
    k = context @ k_w + k_b           # [B,C,E]
    v = context @ v_w + v_b           # [B,C,E]
    attn = softmax(q.k^T / sqrt(hd))  # per head
    out = (attn @ v) @ o_w + o_b      # [B,S,E]

Sharding: 8 cores = 4 batches x 2 head-groups (8 heads = 512 embed cols each).
Each core computes the full attention for its (batch, head-group) and a
partial out-projection; the host sums the two head-group partials per batch
(the "all-reduce") and adds o_b.

Device layout: everything is computed in a transposed orientation so the only
on-device transpose is a cheap [128,128] PE transpose per attention block.
The host passes x^T and context^T; the projections produce Q^T/K^T with the
head dim on partitions and V in natural layout.  Scores are computed
transposed (S^T = K @ Q^T, contraction over hd=64, two heads packed into the
128-row PE array via row groups), a c-chunk PAIR at a time into one 2-bank
[128,1024] psum tile so a single exp covers 1024 columns (amortizes the
~185ns per-instruction ACT access overhead).

The attention@V matmul runs in the cheap orientation: stationary = a
[c=128, s=128] block of P^T (the exp output), moving = V [c=128, 65] (64 head
dims + a ones column that yields the softmax denominator), output [s=128,
130] for a head pair accumulating over c.  This charges only 65 PE rows per
instruction instead of 512 (the cost model charges by moving-free size),
halving attention@V cost versus the [d, s] orientation.  The normalization is
then a per-partition scalar multiply (reciprocal of the denominator columns),
and a PE transpose (fp16 identity, 128 rows, output parked in unused columns
of the ov psum tile) restores the [d, s] layout the out-projection needs.
All matmul operands are fp16 (full-rate on the PE at any moving width; fp32r
is 4x penalized below 256-wide moving operands); the output partials are
stored fp16 (halves the serial out-DMA traffic) and summed fp32 on the host.

Scheduling: DMA transfers serialize on one pipe in transfer-issue order, so
the prologue issues deadline-ordered mega-DMAs (kw+ctx1+kb, qw+qb, x0, then
ctx2+vw) and defers compute-gated DMAs so they never block the queue.  The
emission is software-pipelined: the attention@V + normalize + transpose
chain for the head pair TWO windows back (catching up to one window in
s-tile 2), the Q-projection for s-tile n+1 (steps 1-16), the out-projection
for s-tile n-1 (steps 13-32), and on s-tile 0 the phased K^T/V/Q^T prologue
projections are all interleaved into each head pair's score/exp steps to
keep the PE fed while the scalar engine works through the exps.
"""

import sys

sys.path.insert(0, "/opt/trn_rl_repo")

import numpy as np

B, S, E, C, D = 4, 2048, 1024, 1024, 768
H, HD = 16, 64
EL = E // 2          # embed columns per head-group (8 heads)
N_CORES = 8
NS = S // 512        # s-tiles of 512
KE = E // 128        # contraction chunks for q-proj
KD = D // 128        # contraction chunks for k/v-proj
NC2 = C // 512       # c-tiles of 512
CC = C // 128        # c chunks of 128
HP = EL // 128       # head pairs per core (4)

# "fp32r" (fp22 multiply) or "fp16" (halves DMA traffic and SBUF, and is
# full-rate on the PE for narrow moving operands, which fp32r is not)
DTYPE_MODE = "fp16"
# scores matmul in fp8e4m3 DoubleRow mode (2x PE throughput on the scores).
# Disabled: with the pipelined schedule the kernel is ACT/latency-bound, so
# fp8 scores only bought ~1.3% while costing 30x the accuracy margin (and the
# fp8 path produced NaNs on the interpreter run).
SCORES_FP8 = False

_built = None
_last_results = None


def _build(reps=1, nop_us=0, mode=None):
    import concourse.bacc as bacc
    import concourse.mybir as mybir
    from concourse.tile import TileContext

    F32 = mybir.dt.float32
    F32R = mybir.dt.float32r
    F16 = mybir.dt.float16
    F8 = mybir.dt.float8e4
    DR = mybir.MatmulPerfMode.DoubleRow
    Exp = mybir.ActivationFunctionType.Exp

    if mode is None:
        mode = DTYPE_MODE
    CT = F32R if mode == "fp32r" else F16   # compute dtype for matmul operands
    IN = F32 if mode == "fp32r" else F16    # dram dtype for matmul inputs

    nc = bacc.Bacc(None, target_bir_lowering=False)

    xT = nc.declare_dram_parameter("xT", [E, S], IN, isOutput=False)
    ctxT = nc.declare_dram_parameter("ctxT", [D, C], IN, isOutput=False)
    qw = nc.declare_dram_parameter("qw", [E, EL], IN, isOutput=False)
    kw = nc.declare_dram_parameter("kw", [D, EL], IN, isOutput=False)
    vw = nc.declare_dram_parameter("vw", [D, EL], IN, isOutput=False)
    ow = nc.declare_dram_parameter("ow", [EL, E], IN, isOutput=False)
    qb = nc.declare_dram_parameter("qb", [EL, 1], F32, isOutput=False)
    kb = nc.declare_dram_parameter("kb", [EL, 1], F32, isOutput=False)
    vb = nc.declare_dram_parameter("vb", [1, EL], IN, isOutput=False)
    ones_r = nc.declare_dram_parameter("ones_r", [1, 128], IN, isOutput=False)
    ident = nc.declare_dram_parameter("ident", [128, 128], F16, isOutput=False)
    out = nc.declare_dram_parameter("out", [S, E], F16, isOutput=True)

    def r(ap):
        return ap.bitcast(F32R) if mode == "fp32r" else ap

    with TileContext(nc) as tc:
        with (
            tc.tile_pool(name="wpool", bufs=1) as wpool,
            tc.tile_pool(name="dpool", bufs=1) as dpool,
            tc.tile_pool(name="xpool", bufs=4) as xpool,
            tc.tile_pool(name="qtpool", bufs=8) as qtpool,
            tc.tile_pool(name="ptpool", bufs=32) as ptpool,
            tc.tile_pool(name="ntpool", bufs=6) as ntpool,
            tc.tile_pool(name="otpool", bufs=8) as otpool,
            tc.tile_pool(name="spool", bufs=4) as spool,
            tc.tile_pool(name="opool", bufs=2) as opool,
            tc.tile_pool(name="pspool", bufs=1, space="PSUM") as pspool,
        ):
          for _rep in range(reps):
            # ---- weight / bias / context loads ---------------------------
            # One strided mega-DMA per tensor (chunks packed side-by-side in
            # a single SBUF tile, per-chunk views sliced out): each dma_start
            # holds the global HWDGE issue slot ~625ns, so fewer+bigger wins
            # (per-chunk DMAs serialize on the issue path and delay the whole
            # prologue far more than the coarser dependency costs).
            # Ordered by first use: kw+ctx(first half)+kb -> vw+ctx2+vb -> qw
            # -> x0 -> rest.
            def chunked_tile(pool, nchunk, width, name):
                t = pool.tile([128, nchunk * width], CT, name=name)
                return t, [t[:, i * width:(i + 1) * width] for i in range(nchunk)]

            def load_mega(t, src, nchunk, width):
                nc.sync.dma_start(
                    out=t.rearrange("p (c w) -> p c w", w=width),
                    in_=src.rearrange("(c p) w -> p c w", p=128),
                )

            # DMA issue order = serial transfer order; ordered by deadline:
            # kt(t2=0) needs kw+ctx1+kb; q-proj(0) needs qw+qb+x0; then the
            # second context half / V-projection / out-proj weights.
            kw_t, kw_sb = chunked_tile(wpool, KD, EL, "kw_all")
            vw_t, vw_sb = chunked_tile(wpool, KD, EL, "vw_all")
            ctx_all = dpool.tile([128, KD * C], CT, name="ctx_all")
            ctx_sb = [ctx_all[:, d * C:(d + 1) * C] for d in range(KD)]
            ctx_3d = ctx_all.rearrange("p (c w) -> p c w", w=C)
            load_mega(kw_t, r(kw), KD, EL)
            nc.sync.dma_start(
                out=ctx_3d[:, :, 0:512],
                in_=r(ctxT)[:, 0:512].rearrange("(c p) w -> p c w", p=128),
            )
            kb_t = wpool.tile([128, HP], F32, name="kb_t")
            nc.sync.dma_start(
                out=kb_t.rearrange("p (c w) -> p c w", w=1),
                in_=kb.rearrange("(c p) w -> p c w", p=128),
            )
            kb_sb = [kb_t[:, m:m + 1] for m in range(HP)]
            # act-table prewarm: a dummy exp as soon as kb lands loads the
            # Exp LUT (1.28us) while the PE still waits on weight DMAs
            warm_t = wpool.tile([128, 4], F32, name="warm_t")
            nc.scalar.activation(warm_t[:], kb_t[:], Exp)
            qw_t, qw_sb = chunked_tile(wpool, KE, EL, "qw_all")
            load_mega(qw_t, r(qw), KE, EL)
            qb_t = wpool.tile([128, HP], F32, name="qb_t")
            nc.sync.dma_start(
                out=qb_t.rearrange("p (c w) -> p c w", w=1),
                in_=qb.rearrange("(c p) w -> p c w", p=128),
            )
            qb_sb = [qb_t[:, m:m + 1] for m in range(HP)]

            def load_late_weights():
                nc.sync.dma_start(
                    out=ctx_3d[:, :, 512:1024],
                    in_=r(ctxT)[:, 512:1024]
                    .rearrange("(c p) w -> p c w", p=128),
                )
                load_mega(vw_t, r(vw), KD, EL)
                nc.sync.dma_start(out=vb_sb[:], in_=r(vb[:]))
                nc.sync.dma_start(out=ones_sb[:], in_=r(ones_r[:]))

            vb_sb = wpool.tile([1, EL], CT, name="vb_sb")
            ones_sb = wpool.tile([1, 128], CT, name="ones_sb")
            vb_bc = wpool.tile([128, EL], F32, name="vb_bc")

            def vb_chain():
                vb_ps = pspool.tile([128, 512], F32, name="acc_ps",
                                    tag="acc", bufs=2)
                nc.tensor.matmul(vb_ps[:], ones_sb[0:1, :], vb_sb[:],
                                 start=True, stop=True)
                nc.vector.tensor_copy(vb_bc[:], vb_ps[:])

            ident_sb = wpool.tile([128, 128], F16, name="ident_sb")
            ow_all = wpool.tile([128, HP * E], CT, name="ow_all")
            ow_sb = [ow_all[:, k * E:(k + 1) * E] for k in range(HP)]

            def load_ow():
                nc.sync.dma_start(out=ident_sb[:], in_=ident[:])
                nc.sync.dma_start(
                    out=ow_all.rearrange("p (c w) -> p c w", w=E),
                    in_=r(ow).rearrange("(c p) w -> p c w", p=128),
                )

            # ---- K^T projection: [EL rows, C cols], head pairs on partitions --
            # With SCORES_FP8 the epilogue writes fp8e4m3 and a small SBUF
            # DMA relayouts each head's [64, C] half into [32, 2, C] (the
            # hd-dim split across the free dim) for the DoubleRow matmul.
            ST = F8 if SCORES_FP8 else CT
            kt_sb = []
            kt8v = []
            for m in range(HP):
                t = dpool.tile([128, C], ST, name=f"kt{m}")
                kt_sb.append(t)
                if SCORES_FP8:
                    kt8v.append([
                        dpool.tile([32, 2 * C], F8, name=f"kt8v{m}_{h2}")
                        .rearrange("p (g c) -> p g c", c=C)
                        for h2 in range(2)])

            def kt_relayout(m, t2):
                for h2 in range(2):
                    nc.sync.dma_start(
                        out=kt8v[m][h2][:, :, t2 * 512:(t2 + 1) * 512],
                        in_=kt_sb[m][h2 * 64:(h2 + 1) * 64,
                                     t2 * 512:(t2 + 1) * 512]
                        .rearrange("(g p) c -> p g c", p=32),
                    )

            def kt_thunks(m, t2s=range(NC2)):
                """Matmul thunks computing K^T halves for head pair m."""
                state = {}
                thunks = []

                def f(t2, d):
                    if d == 0:
                        state[t2] = pspool.tile(
                            [128, 512], F32, name="acc_ps", tag="acc", bufs=2)
                    ps = state[t2]
                    nc.tensor.matmul(
                        ps[:],
                        kw_sb[d][:, m * 128:(m + 1) * 128],
                        ctx_sb[d][:, t2 * 512:(t2 + 1) * 512],
                        start=(d == 0), stop=(d == KD - 1),
                    )
                    if d == KD - 1:
                        with nc.allow_low_precision("fp8 scores"):
                            nc.vector.tensor_scalar_add(
                                kt_sb[m][:, t2 * 512:(t2 + 1) * 512], ps[:],
                                kb_sb[m][:, 0:1],
                            )
                        if SCORES_FP8:
                            if defer_dma is not None:
                                defer_dma.append((kt_relayout, m, t2))
                            else:
                                kt_relayout(m, t2)

                for t2 in t2s:
                    for d in range(KD):
                        thunks.append((f, t2, d))
                return thunks

            # ---- V projection: natural [C rows, EL cols], interleaved with a
            # ones column per head for the softmax denominator ------------------
            v_sb = []
            for mc in range(CC):
                t = dpool.tile([128, 8 * 65], CT, name=f"v{mc}")
                v_sb.append(t)

            def vproj_group(mc):
                t = v_sb[mc]
                ps = pspool.tile([128, 512], F32, name="acc_ps", tag="acc", bufs=2)
                for d in range(KD):
                    nc.tensor.matmul(
                        ps[:],
                        ctx_sb[d][:, mc * 128:(mc + 1) * 128],
                        vw_sb[d][:],
                        start=(d == 0), stop=(d == KD - 1),
                    )
                vv = t.rearrange("p (h u) -> p h u", u=65)
                nc.vector.tensor_add(
                    vv[:, :, 0:64],
                    ps.rearrange("p (h u) -> p h u", u=64),
                    vb_bc.rearrange("p (h u) -> p h u", u=64),
                )
                nc.vector.tensor_scalar(
                    vv[:, :, 64:65],
                    vb_bc[:, 0:8].rearrange("p (h u) -> p h u", u=1),
                    0.0, 1.0,
                    mybir.AluOpType.mult, mybir.AluOpType.add,
                )  # writes the constant 1.0 column

            # ---- pipelined main loop over s-tiles of 512 ----------------------
            xts_all = {}
            qts_all = {}
            ots_all = {}
            pts_all = {}

            def load_x(n):
                tiles = []
                for half in range(2):
                    t = xpool.tile([128, 4 * 512], CT, name="xt", tag="xt")
                    views = [t[:, i * 512:(i + 1) * 512] for i in range(4)]
                    nc.sync.dma_start(
                        out=t.rearrange("p (c w) -> p c w", w=512),
                        in_=r(xT[half * 512:(half + 1) * 512,
                                 n * 512:(n + 1) * 512])
                        .rearrange("(c p) w -> p c w", p=128),
                    )
                    tiles += views
                xts_all[n] = tiles

            def qt_relayout(n, m, qt_t):
                v8 = []
                for h2 in range(2):
                    t8 = qtpool.tile([32, 1024], F8, name="qt8v",
                                     tag="qt8v", bufs=16)
                    nc.sync.dma_start(
                        out=t8.rearrange("p (g s) -> p g s", s=512),
                        in_=qt_t[h2 * 64:(h2 + 1) * 64, :]
                        .rearrange("(g p) s -> p g s", p=32),
                    )
                    v8.append(t8.rearrange("p (g s) -> p g s", s=512))
                qts_all[n][m] = v8

            def qproj_thunks(n):
                """32 matmul thunks computing Q^T for s-tile n (4 psum groups)."""
                state = {}
                thunks = []
                qts_all[n] = [None] * HP

                def f(m, k):
                    if k == 0:
                        state[m] = pspool.tile(
                            [128, 512], F32, name="acc_ps", tag="acc", bufs=2)
                    ps = state[m]
                    nc.tensor.matmul(
                        ps[:],
                        qw_sb[k][:, m * 128:(m + 1) * 128],
                        xts_all[n][k][:],
                        start=(k == 0), stop=(k == KE - 1),
                    )
                    if k == KE - 1:
                        qt_t = qtpool.tile([128, 512], ST, name="qt", tag="qt")
                        with nc.allow_low_precision("fp8 scores"):
                            nc.vector.tensor_scalar_add(
                                qt_t[:], ps[:], qb_sb[m][:, 0:1])
                        if SCORES_FP8:
                            if defer_dma is not None:
                                defer_dma.append((qt_relayout, n, m, qt_t))
                            else:
                                qt_relayout(n, m, qt_t)
                        else:
                            qts_all[n][m] = qt_t

                for m in range(HP):
                    for k in range(KE):
                        thunks.append((f, m, k))
                return thunks

            def outproj_thunks(n):
                """32 matmul thunks for the out-projection of s-tile n."""
                state = {}
                thunks = []

                def f(ss, ne, hp):
                    if hp == 0:
                        state[(ss, ne)] = pspool.tile(
                            [128, 512], F32, name="acc_ps", tag="acc", bufs=2)
                        if ne == 0:
                            state[ss] = opool.tile(
                                [128, 1024], F16, name="o_sb", tag="o")
                    ps = state[(ss, ne)]
                    nc.tensor.matmul(
                        ps[:],
                        ots_all[n][hp][:, ss * 128:(ss + 1) * 128],
                        ow_sb[hp][:, ne * 512:(ne + 1) * 512],
                        start=(hp == 0), stop=(hp == HP - 1),
                    )
                    if hp == HP - 1:
                        o_sb = state[ss]
                        nc.vector.tensor_copy(
                            o_sb[:, ne * 512:(ne + 1) * 512], ps[:])
                        # store each half as soon as its copy lands: smaller
                        # final flush, earlier overlap on the serial DMA pipe
                        nc.sync.dma_start(
                            out=out[n * 512 + ss * 128:
                                    n * 512 + (ss + 1) * 128,
                                    ne * 512:(ne + 1) * 512],
                            in_=o_sb[:, ne * 512:(ne + 1) * 512],
                        )

                for ss in range(4):
                    for ne in range(2):
                        for hp in range(HP):
                            thunks.append((f, ss, ne, hp))
                return thunks

            def make_av(n, hp):
                """builders for the attention@V + normalize + transpose
                chain for (n, hp).

                Per s-block sb: 16 matmuls accumulate [s=128, 65]x2 heads into
                one [128,130] psum (col 64 / 129 = softmax denominators from
                the ones column of V), then reciprocal + 2 per-partition
                scalar multiplies normalize into an SBUF tile, and a PE
                transpose (fp16 identity) yields the [d, s] block the
                out-projection consumes.

                Returns (thunks, carry): the normalize/transpose chain lags
                the matmuls by one s-block, and the last block's chain is
                returned as `carry` to be drained at the start of the next
                head pair's steps — the PE transpose sits in the PE stream
                and would otherwise stall it on the DVE norm latency.
                """
                state = {}

                def mm(sb, h2, c):
                    pts = pts_all[(n, hp)]
                    # two s-blocks' [128,130] accumulators packed per psum
                    # bank (regions at col 0 and 256)
                    if c == 0 and h2 == 0 and sb % 2 == 0:
                        state[sb // 2] = pspool.tile(
                            [128, 512], F32, name="ov_ps", tag="ov", bufs=2)
                    ps = state[sb // 2]
                    base = (sb % 2) * 256
                    h = hp * 2 + h2
                    nc.tensor.matmul(
                        ps[:, base + h2 * 65:base + (h2 + 1) * 65],
                        pts[(c // 2, h2)][:, (c % 2) * 512
                                          + sb * 128:(c % 2) * 512
                                          + (sb + 1) * 128],
                        v_sb[c][:, h * 65:(h + 1) * 65],
                        start=(c == 0), stop=(c == CC - 1),
                    )

                def norm(sb):
                    ps = state[sb // 2]
                    base = (sb % 2) * 256
                    rs = spool.tile([128, 2], F32, name="rs", tag="rs")
                    with nc.allow_low_precision("softmax denom"):
                        nc.vector.reciprocal(
                            rs.rearrange("p (g u) -> p g u", u=1),
                            ps[:, base:base + 130]
                            .rearrange("p (g u) -> p g u", u=65)[:, :, 64:65])
                    nt = ntpool.tile([128, 128], F16, name="nt", tag="nt")
                    nc.vector.tensor_scalar_mul(
                        nt[:, 0:64], ps[:, base:base + 64], rs[:, 0:1])
                    nc.vector.tensor_scalar_mul(
                        nt[:, 64:128], ps[:, base + 65:base + 129], rs[:, 1:2])
                    state[(sb, "nt")] = nt

                def transp(sb):
                    # transpose output parks in the unused columns of the
                    # already-allocated ov pair tile (as an fp16 view) — no
                    # extra psum bank, no allocation to wait on
                    nt = state[(sb, "nt")]
                    ps = state[sb // 2]
                    base_tr = (sb % 2) * 256 + 136
                    tr = ps[:, base_tr:base_tr + 64].bitcast(F16)
                    nc.tensor.transpose(tr, nt[:], ident_sb[:])
                    state[(sb, "tr")] = tr

                def trcopy(sb):
                    if sb == 0:
                        ots_all[n][hp] = otpool.tile(
                            [128, 512], CT, name="ot", tag="ot")
                    tr = state[(sb, "tr")]
                    nc.vector.tensor_copy(
                        ots_all[n][hp][:, sb * 128:(sb + 1) * 128], tr)

                def ntc(sb):
                    return [(norm, sb), (transp, sb), (trcopy, sb)]

                return mm, ntc

            def av_thunks(n, hp):
                mm, ntc = make_av(n, hp)

                def mms(sb):
                    return [(mm, sb, h2, c) for h2 in range(2) for c in range(CC)]

                thunks = (mms(0) + mms(1) + ntc(0) + mms(2) + ntc(1)
                          + mms(3) + ntc(2))
                return thunks, ntc(3)

            def run_thunks(ts):
                for f, *args in ts:
                    f(*args)

            # prologue, ordered to match serial DMA arrival (kw+ctx1+kb, qw,
            # x0, vw+ctx2, ow): K^T m=0 first half inline, Q^T(0) m=0 inline;
            # everything else (K^T second half + other head pairs, the whole
            # V projection, remaining Q^T(0) groups) rides in attention(0)'s
            # background, phased by deadline: each head pair's K^T/Q^T lands
            # before the head pair that needs it, V before attention@V(0,0).
            load_x(0)
            defer_dma = []
            run_thunks(kt_thunks(0, t2s=[0]))
            run_thunks(kt_thunks(1, t2s=[0]))   # keeps the PE warm until x0
            qp0 = qproj_thunks(0)
            run_thunks(qp0[:KE])          # m=0 group
            load_late_weights()
            # the deferred fp8 relayouts issue AFTER ctx2/vw so their
            # compute-gated sem waits don't block the late weights in the
            # serial DMA queue
            for fdma, *fargs in defer_dma:
                fdma(*fargs)
            defer_dma = None
            prologue_phases = [
                # kt(0,t2=1) must fully drain before step 5 emits the first
                # c>=4 score matmul (reads emitted before their producer are
                # invisible to the dependency tracker)
                (kt_thunks(0, t2s=[1]) + qp0[KE:2 * KE]
                 + kt_thunks(1, t2s=[1])),
                ([(load_ow,), (vb_chain,)]
                 + [(vproj_group, mc) for mc in range(6)]
                 + kt_thunks(2) + qp0[2 * KE:3 * KE]),
                ([(vproj_group, mc) for mc in range(6, CC)]
                 + kt_thunks(3) + qp0[3 * KE:4 * KE]),
                [],
            ]

            carry = []
            for n in range(NS):
                if n + 1 < NS:
                    load_x(n + 1)
                bg1 = qproj_thunks(n + 1) if n + 1 < NS else []
                bg2 = outproj_thunks(n - 1) if n >= 1 else []

                ots_all[n] = [None] * HP
                qts = qts_all[n]
                n_steps = HP * CC
                step = 0
                bg1_done = 0
                bg2_done = 0
                for hp in range(HP):
                    # attention@V drains lag their head pair by TWO windows
                    # through s-tiles 0/1 (s-tile 0 is PE-oversubscribed by
                    # the projection prologue), catch up to a one-window lag
                    # in s-tile 2's slack, so the epilogue owes only one av
                    i = n * HP + hp
                    if i < 2 * HP:
                        js = [i - 2] if i >= 2 else []
                    elif i == 2 * HP:
                        js = [i - 2, i - 1]
                    else:
                        js = [i - 1]
                    a, newcarry = [], []
                    for j in js:
                        aj, cj = av_thunks(j // HP, j % HP)
                        a += aj
                        newcarry += cj
                    if i == NS * HP - 1:
                        # last window: finish av(3,2) entirely in-window so
                        # the epilogue av's psum slots don't wait on a carried
                        # DVE chain
                        a += newcarry
                        newcarry = []
                    abg = carry + a
                    carry = newcarry
                    abg_done = 0
                    ph = prologue_phases[hp] if n == 0 else []
                    ph_done = 0
                    pts = {}
                    pts_all[(n, hp)] = pts
                    for cp in range(CC // 2):
                      for h2 in range(2):
                        # scores^T for a c-chunk PAIR into one 2-bank psum
                        # tile so a single exp covers 1024 columns (the
                        # per-instruction ACT access overhead is ~30% at 512).
                        # K_h @ Q_h^T, contraction hd=64; h2=0 uses PE rows
                        # 0-63, h2=1 rows 64-127 (row groups).
                        sc = pspool.tile(
                            [128, 1024], F32, name="sc_ps", tag="sc", bufs=2)
                        for ci in range(2):
                            c = 2 * cp + ci
                            if SCORES_FP8:
                                for sh in range(2):
                                    nc.tensor.matmul(
                                        sc[:, ci * 512 + sh * 256:
                                           ci * 512 + (sh + 1) * 256],
                                        kt8v[hp][h2][:, :,
                                                     c * 128:(c + 1) * 128],
                                        qts[hp][h2][:, :,
                                                    sh * 256:(sh + 1) * 256],
                                        start=True, stop=True,
                                        perf_mode=DR,
                                    )
                            else:
                                nc.tensor.matmul(
                                    sc[:, ci * 512:(ci + 1) * 512],
                                    kt_sb[hp][h2 * 64:(h2 + 1) * 64,
                                              c * 128:(c + 1) * 128],
                                    qts[hp][h2 * 64:(h2 + 1) * 64, :],
                                    start=True, stop=True,
                                )
                        p = ptpool.tile([128, 1024], CT, name="pt", tag="pt")
                        nc.scalar.activation(p[:], sc[:], Exp)
                        pts[(cp, h2)] = p
                        # inject background work (attention@V chain for hp-1,
                        # q-proj n+1 / out-proj n-1) between the score steps so
                        # the PE stays busy through the exp latency
                        step += 1
                        stepin = cp * 2 + h2 + 1
                        # startup work (s-tile 0) phased by deadline
                        ptarget = min(len(ph), stepin * len(ph) // (CC - 1))
                        while ph_done < ptarget:
                            f, *args = ph[ph_done]
                            f(*args)
                            ph_done += 1
                        atarget = min(len(abg),
                                      stepin * len(abg) // (CC - 1))
                        while abg_done < atarget:
                            f, *args = abg[abg_done]
                            f(*args)
                            abg_done += 1
                        # q-proj(n+1) over steps 1-16; out-proj(n-1) over
                        # steps 13-32 (its hp=2/3 stationaries are produced
                        # by the lagged av drains in this s-tile's first half)
                        t1 = min(len(bg1), step * len(bg1) // 16)
                        while bg1_done < t1:
                            f, *args = bg1[bg1_done]
                            f(*args)
                            bg1_done += 1
                        t2 = (0 if step <= 12 else
                              min(len(bg2),
                                  (step - 12) * len(bg2) // (n_steps - 12)))
                        while bg2_done < t2:
                            f, *args = bg2[bg2_done]
                            f(*args)
                            bg2_done += 1
                    run_thunks(ph[ph_done:])
                    run_thunks(abg[abg_done:])
                run_thunks(bg1[bg1_done:])
                run_thunks(bg2[bg2_done:])

            # epilogue: attention@V for the last head pair interleaved with
            # the out-projection of the last s-tile.
            # av layout: [carry][mms0 16][mms1 16][ntc0 3][mms2 16][ntc1 3]
            # [mms3 16][ntc2 3][ntc3 3]; out-proj group ss needs ntc(ss).
            a, newcarry = av_thunks(NS - 1, HP - 1)
            av_all = carry + a + newcarry
            oth = outproj_thunks(NS - 1)
            nca = len(carry)
            cuts = [nca + 35, nca + 54, nca + 73, len(av_all)]
            merged = []
            ai = 0
            for ss in range(4):
                merged += av_all[ai:cuts[ss]]
                merged += oth[ss * 8:(ss + 1) * 8]
                ai = cuts[ss]
            run_thunks(merged)

          # timing aid: calibrated delay chain on the otherwise-idle gpsimd
          # engine; kernel exec time = max(real work, nop chain)
          if nop_us:
            NOP_CYC = 48000  # 40 us at 1.2 GHz
            for _ in range(int(nop_us * 1200 / NOP_CYC)):
                nc.gpsimd.nop(cycle_cnt=NOP_CYC, nofuse=True)

    nc.finalize()
    return nc


def kernel(x, context, q_w, q_b, k_w, k_b, v_w, v_b, o_w, o_b):
    global _built, _last_results
    from concourse.bass_utils import run_bass_kernel_spmd

    if _built is None:
        _built = _build()
    nc = _built

    scale = np.float32(1.0 / np.sqrt(HD))
    ind = np.float32 if DTYPE_MODE == "fp32r" else np.float16
    x = np.asarray(x, np.float32)
    context = np.asarray(context, np.float32)
    xTs = [np.ascontiguousarray(x[b].T).astype(ind) for b in range(B)]
    ctxTs = [np.ascontiguousarray(context[b].T).astype(ind) for b in range(B)]

    in_maps = []
    for core in range(N_CORES):
        b, hg = core // 2, core % 2
        el = slice(hg * EL, (hg + 1) * EL)
        in_maps.append({
            "xT": xTs[b],
            "ctxT": ctxTs[b],
            "qw": np.ascontiguousarray(
                (np.asarray(q_w, np.float32)[:, el] * scale).astype(ind)),
            "kw": np.ascontiguousarray(np.asarray(k_w, np.float32)[:, el]).astype(ind),
            "vw": np.ascontiguousarray(np.asarray(v_w, np.float32)[:, el]).astype(ind),
            "ow": np.ascontiguousarray(np.asarray(o_w, np.float32)[el, :]).astype(ind),
            "qb": np.ascontiguousarray(
                (np.asarray(q_b, np.float32)[el] * scale)[:, None]),
            "kb": np.ascontiguousarray(np.asarray(k_b, np.float32)[el][:, None]),
            "vb": np.ascontiguousarray(
                np.asarray(v_b, np.float32)[el][None, :]).astype(ind),
            "ones_r": np.ones((1, 128), ind),
            "ident": np.eye(128, dtype=np.float16),
        })

    res = run_bass_kernel_spmd(nc, in_maps, list(range(N_CORES)))
    _last_results = res

    ob = np.asarray(o_b, np.float32)
    full = np.empty((B, S, E), np.float32)
    for b in range(B):
        full[b] = (res.results[2 * b]["out"].astype(np.float32)
                   + res.results[2 * b + 1]["out"].astype(np.float32) + ob)
    return full


# revision 65
# speedup vs baseline: 1.2566x; 1.0120x over previous
"""Cross-attention kernel for Trainium2, 8 NeuronCores.

Reference computation (B=4, S=2048, C=1024, E=1024, D=768, H=16, hd=64):
    q = x @ q_w + q_b                 # [B,S,E]
    k = context @ k_w + k_b           # [B,C,E]
    v = context @ v_w + v_b           # [B,C,E]
    attn = softmax(q.k^T / sqrt(hd))  # per head
    out = (attn @ v) @ o_w + o_b      # [B,S,E]

Sharding: 8 cores = 4 batches x 2 head-groups (8 heads = 512 embed cols each).
Each core computes the full attention for its (batch, head-group) and a
partial out-projection; the host sums the two head-group partials per batch
(the "all-reduce") and adds o_b.

Device layout: everything is computed in a transposed orientation so the only
on-device transpose is a cheap [128,128] PE transpose per attention block.
The host passes x^T and context^T; the projections produce Q^T/K^T with the
head dim on partitions and V in natural layout.  Scores are computed
transposed (S^T = K @ Q^T, contraction over hd=64, two heads packed into the
128-row PE array via row groups), a c-chunk PAIR at a time into one 2-bank
[128,1024] psum tile so a single exp covers 1024 columns (amortizes the
~185ns per-instruction ACT access overhead).

The attention@V matmul runs in the cheap orientation: stationary = a
[c=128, s=128] block of P^T (the exp output), moving = V [c=128, 65] (64 head
dims + a ones column that yields the softmax denominator), output [s=128,
130] for a head pair accumulating over c.  This charges only 65 PE rows per
instruction instead of 512 (the cost model charges by moving-free size),
halving attention@V cost versus the [d, s] orientation.  The normalization is
then a per-partition scalar multiply (reciprocal of the denominator columns),
and a PE transpose (fp16 identity, 128 rows, output parked in unused columns
of the ov psum tile) restores the [d, s] layout the out-projection needs.
All matmul operands are fp16 (full-rate on the PE at any moving width; fp32r
is 4x penalized below 256-wide moving operands); the output partials are
stored fp16 (halves the serial out-DMA traffic) and summed fp32 on the host.

Scheduling: DMA transfers serialize on one pipe in transfer-issue order, so
the prologue issues deadline-ordered mega-DMAs (kw+ctx1+kb, qw+qb, x0, then
ctx2+vw) and defers compute-gated DMAs so they never block the queue.  The
emission is software-pipelined: the attention@V + normalize + transpose
chain for the head pair TWO windows back (catching up to one window in
s-tile 2), the Q-projection for s-tile n+1 (steps 1-16), the out-projection
for s-tile n-1 (steps 13-32), and on s-tile 0 the phased K^T/V/Q^T prologue
projections are all interleaved into each head pair's score/exp steps to
keep the PE fed while the scalar engine works through the exps.
"""

import sys

sys.path.insert(0, "/opt/trn_rl_repo")

import numpy as np

B, S, E, C, D = 4, 2048, 1024, 1024, 768
H, HD = 16, 64
EL = E // 2          # embed columns per head-group (8 heads)
N_CORES = 8
NS = S // 512        # s-tiles of 512
KE = E // 128        # contraction chunks for q-proj
KD = D // 128        # contraction chunks for k/v-proj
NC2 = C // 512       # c-tiles of 512
CC = C // 128        # c chunks of 128
HP = EL // 128       # head pairs per core (4)

# "fp32r" (fp22 multiply) or "fp16" (halves DMA traffic and SBUF, and is
# full-rate on the PE for narrow moving operands, which fp32r is not)
DTYPE_MODE = "fp16"
# scores matmul in fp8e4m3 DoubleRow mode (2x PE throughput on the scores).
# Disabled: with the pipelined schedule the kernel is ACT/latency-bound, so
# fp8 scores only bought ~1.3% while costing 30x the accuracy margin (and the
# fp8 path produced NaNs on the interpreter run).
SCORES_FP8 = False

_built = None
_last_results = None


def _build(reps=1, nop_us=0, mode=None):
    import concourse.bacc as bacc
    import concourse.mybir as mybir
    from concourse.tile import TileContext

    F32 = mybir.dt.float32
    F32R = mybir.dt.float32r
    F16 = mybir.dt.float16
    F8 = mybir.dt.float8e4
    DR = mybir.MatmulPerfMode.DoubleRow
    Exp = mybir.ActivationFunctionType.Exp

    if mode is None:
        mode = DTYPE_MODE
    CT = F32R if mode == "fp32r" else F16   # compute dtype for matmul operands
    IN = F32 if mode == "fp32r" else F16    # dram dtype for matmul inputs

    nc = bacc.Bacc(None, target_bir_lowering=False)

    xT = nc.declare_dram_parameter("xT", [E, S], IN, isOutput=False)
    ctxT = nc.declare_dram_parameter("ctxT", [D, C], IN, isOutput=False)
    qw = nc.declare_dram_parameter("qw", [E, EL], IN, isOutput=False)
    kw = nc.declare_dram_parameter("kw", [D, EL], IN, isOutput=False)
    vw = nc.declare_dram_parameter("vw", [D, EL], IN, isOutput=False)
    ow = nc.declare_dram_parameter("ow", [EL, E], IN, isOutput=False)
    qb = nc.declare_dram_parameter("qb", [EL, 1], F32, isOutput=False)
    kb = nc.declare_dram_parameter("kb", [EL, 1], F32, isOutput=False)
    vb = nc.declare_dram_parameter("vb", [1, EL], IN, isOutput=False)
    ones_r = nc.declare_dram_parameter("ones_r", [1, 128], IN, isOutput=False)
    ident = nc.declare_dram_parameter("ident", [128, 128], F16, isOutput=False)
    out = nc.declare_dram_parameter("out", [S, E], F16, isOutput=True)

    def r(ap):
        return ap.bitcast(F32R) if mode == "fp32r" else ap

    with TileContext(nc) as tc:
        with (
            tc.tile_pool(name="wpool", bufs=1) as wpool,
            tc.tile_pool(name="dpool", bufs=1) as dpool,
            tc.tile_pool(name="xpool", bufs=4) as xpool,
            tc.tile_pool(name="qtpool", bufs=8) as qtpool,
            tc.tile_pool(name="ptpool", bufs=32) as ptpool,
            tc.tile_pool(name="ntpool", bufs=6) as ntpool,
            tc.tile_pool(name="otpool", bufs=8) as otpool,
            tc.tile_pool(name="spool", bufs=4) as spool,
            tc.tile_pool(name="opool", bufs=2) as opool,
            tc.tile_pool(name="pspool", bufs=1, space="PSUM") as pspool,
        ):
          for _rep in range(reps):
            # ---- weight / bias / context loads ---------------------------
            # One strided mega-DMA per tensor (chunks packed side-by-side in
            # a single SBUF tile, per-chunk views sliced out): each dma_start
            # holds the global HWDGE issue slot ~625ns, so fewer+bigger wins
            # (per-chunk DMAs serialize on the issue path and delay the whole
            # prologue far more than the coarser dependency costs).
            # Ordered by first use: kw+ctx(first half)+kb -> vw+ctx2+vb -> qw
            # -> x0 -> rest.
            def chunked_tile(pool, nchunk, width, name):
                t = pool.tile([128, nchunk * width], CT, name=name)
                return t, [t[:, i * width:(i + 1) * width] for i in range(nchunk)]

            def load_mega(t, src, nchunk, width):
                nc.sync.dma_start(
                    out=t.rearrange("p (c w) -> p c w", w=width),
                    in_=src.rearrange("(c p) w -> p c w", p=128),
                )

            # DMA issue order = serial transfer order; ordered by deadline:
            # kt(t2=0) needs kw+ctx1+kb; q-proj(0) needs qw+qb+x0; then the
            # second context half / V-projection / out-proj weights.
            kw_t, kw_sb = chunked_tile(wpool, KD, EL, "kw_all")
            vw_t, vw_sb = chunked_tile(wpool, KD, EL, "vw_all")
            ctx_all = dpool.tile([128, KD * C], CT, name="ctx_all")
            ctx_sb = [ctx_all[:, d * C:(d + 1) * C] for d in range(KD)]
            ctx_3d = ctx_all.rearrange("p (c w) -> p c w", w=C)
            load_mega(kw_t, r(kw), KD, EL)
            nc.sync.dma_start(
                out=ctx_3d[:, :, 0:512],
                in_=r(ctxT)[:, 0:512].rearrange("(c p) w -> p c w", p=128),
            )
            kb_t = wpool.tile([128, HP], F32, name="kb_t")
            nc.sync.dma_start(
                out=kb_t.rearrange("p (c w) -> p c w", w=1),
                in_=kb.rearrange("(c p) w -> p c w", p=128),
            )
            kb_sb = [kb_t[:, m:m + 1] for m in range(HP)]
            # act-table prewarm: a dummy exp as soon as kb lands loads the
            # Exp LUT (1.28us) while the PE still waits on weight DMAs
            warm_t = wpool.tile([128, 4], F32, name="warm_t")
            nc.scalar.activation(warm_t[:], kb_t[:], Exp)
            qw_t, qw_sb = chunked_tile(wpool, KE, EL, "qw_all")
            load_mega(qw_t, r(qw), KE, EL)
            qb_t = wpool.tile([128, HP], F32, name="qb_t")
            nc.sync.dma_start(
                out=qb_t.rearrange("p (c w) -> p c w", w=1),
                in_=qb.rearrange("(c p) w -> p c w", p=128),
            )
            qb_sb = [qb_t[:, m:m + 1] for m in range(HP)]

            def load_late_weights():
                nc.sync.dma_start(
                    out=ctx_3d[:, :, 512:1024],
                    in_=r(ctxT)[:, 512:1024]
                    .rearrange("(c p) w -> p c w", p=128),
                )
                load_mega(vw_t, r(vw), KD, EL)
                nc.sync.dma_start(out=vb_sb[:], in_=r(vb[:]))
                nc.sync.dma_start(out=ones_sb[:], in_=r(ones_r[:]))

            vb_sb = wpool.tile([1, EL], CT, name="vb_sb")
            ones_sb = wpool.tile([1, 128], CT, name="ones_sb")
            vb_bc = wpool.tile([128, EL], F32, name="vb_bc")

            def vb_chain():
                vb_ps = pspool.tile([128, 512], F32, name="acc_ps",
                                    tag="acc", bufs=2)
                nc.tensor.matmul(vb_ps[:], ones_sb[0:1, :], vb_sb[:],
                                 start=True, stop=True)
                nc.vector.tensor_copy(vb_bc[:], vb_ps[:])

            ident_sb = wpool.tile([128, 128], F16, name="ident_sb")
            ow_all = wpool.tile([128, HP * E], CT, name="ow_all")
            ow_sb = [ow_all[:, k * E:(k + 1) * E] for k in range(HP)]

            def load_ow():
                nc.sync.dma_start(out=ident_sb[:], in_=ident[:])
                nc.sync.dma_start(
                    out=ow_all.rearrange("p (c w) -> p c w", w=E),
                    in_=r(ow).rearrange("(c p) w -> p c w", p=128),
                )

            # ---- K^T projection: [EL rows, C cols], head pairs on partitions --
            # With SCORES_FP8 the epilogue writes fp8e4m3 and a small SBUF
            # DMA relayouts each head's [64, C] half into [32, 2, C] (the
            # hd-dim split across the free dim) for the DoubleRow matmul.
            ST = F8 if SCORES_FP8 else CT
            kt_sb = []
            kt8v = []
            for m in range(HP):
                t = dpool.tile([128, C], ST, name=f"kt{m}")
                kt_sb.append(t)
                if SCORES_FP8:
                    kt8v.append([
                        dpool.tile([32, 2 * C], F8, name=f"kt8v{m}_{h2}")
                        .rearrange("p (g c) -> p g c", c=C)
                        for h2 in range(2)])

            def kt_relayout(m, t2):
                for h2 in range(2):
                    nc.sync.dma_start(
                        out=kt8v[m][h2][:, :, t2 * 512:(t2 + 1) * 512],
                        in_=kt_sb[m][h2 * 64:(h2 + 1) * 64,
                                     t2 * 512:(t2 + 1) * 512]
                        .rearrange("(g p) c -> p g c", p=32),
                    )

            def kt_thunks(m, t2s=range(NC2)):
                """Matmul thunks computing K^T halves for head pair m."""
                state = {}
                thunks = []

                def f(t2, d):
                    if d == 0:
                        state[t2] = pspool.tile(
                            [128, 512], F32, name="acc_ps", tag="acc", bufs=2)
                    ps = state[t2]
                    nc.tensor.matmul(
                        ps[:],
                        kw_sb[d][:, m * 128:(m + 1) * 128],
                        ctx_sb[d][:, t2 * 512:(t2 + 1) * 512],
                        start=(d == 0), stop=(d == KD - 1),
                    )
                    if d == KD - 1:
                        with nc.allow_low_precision("fp8 scores"):
                            nc.vector.tensor_scalar_add(
                                kt_sb[m][:, t2 * 512:(t2 + 1) * 512], ps[:],
                                kb_sb[m][:, 0:1],
                            )
                        if SCORES_FP8:
                            if defer_dma is not None:
                                defer_dma.append((kt_relayout, m, t2))
                            else:
                                kt_relayout(m, t2)

                for t2 in t2s:
                    for d in range(KD):
                        thunks.append((f, t2, d))
                return thunks

            # ---- V projection: natural [C rows, EL cols], interleaved with a
            # ones column per head for the softmax denominator ------------------
            v_sb = []
            for mc in range(CC):
                t = dpool.tile([128, 8 * 65], CT, name=f"v{mc}")
                v_sb.append(t)

            def vproj_group(mc):
                t = v_sb[mc]
                ps = pspool.tile([128, 512], F32, name="acc_ps", tag="acc", bufs=2)
                for d in range(KD):
                    nc.tensor.matmul(
                        ps[:],
                        ctx_sb[d][:, mc * 128:(mc + 1) * 128],
                        vw_sb[d][:],
                        start=(d == 0), stop=(d == KD - 1),
                    )
                vv = t.rearrange("p (h u) -> p h u", u=65)
                nc.vector.tensor_add(
                    vv[:, :, 0:64],
                    ps.rearrange("p (h u) -> p h u", u=64),
                    vb_bc.rearrange("p (h u) -> p h u", u=64),
                )
                nc.vector.tensor_scalar(
                    vv[:, :, 64:65],
                    vb_bc[:, 0:8].rearrange("p (h u) -> p h u", u=1),
                    0.0, 1.0,
                    mybir.AluOpType.mult, mybir.AluOpType.add,
                )  # writes the constant 1.0 column

            # ---- pipelined main loop over s-tiles of 512 ----------------------
            xts_all = {}
            qts_all = {}
            ots_all = {}
            pts_all = {}

            def load_x(n):
                tiles = []
                for half in range(2):
                    t = xpool.tile([128, 4 * 512], CT, name="xt", tag="xt")
                    views = [t[:, i * 512:(i + 1) * 512] for i in range(4)]
                    nc.sync.dma_start(
                        out=t.rearrange("p (c w) -> p c w", w=512),
                        in_=r(xT[half * 512:(half + 1) * 512,
                                 n * 512:(n + 1) * 512])
                        .rearrange("(c p) w -> p c w", p=128),
                    )
                    tiles += views
                xts_all[n] = tiles

            def qt_relayout(n, m, qt_t):
                v8 = []
                for h2 in range(2):
                    t8 = qtpool.tile([32, 1024], F8, name="qt8v",
                                     tag="qt8v", bufs=16)
                    nc.sync.dma_start(
                        out=t8.rearrange("p (g s) -> p g s", s=512),
                        in_=qt_t[h2 * 64:(h2 + 1) * 64, :]
                        .rearrange("(g p) s -> p g s", p=32),
                    )
                    v8.append(t8.rearrange("p (g s) -> p g s", s=512))
                qts_all[n][m] = v8

            def qproj_thunks(n):
                """32 matmul thunks computing Q^T for s-tile n (4 psum groups)."""
                state = {}
                thunks = []
                qts_all[n] = [None] * HP

                def f(m, k):
                    if k == 0:
                        state[m] = pspool.tile(
                            [128, 512], F32, name="acc_ps", tag="acc", bufs=2)
                    ps = state[m]
                    nc.tensor.matmul(
                        ps[:],
                        qw_sb[k][:, m * 128:(m + 1) * 128],
                        xts_all[n][k][:],
                        start=(k == 0), stop=(k == KE - 1),
                    )
                    if k == KE - 1:
                        qt_t = qtpool.tile([128, 512], ST, name="qt", tag="qt")
                        with nc.allow_low_precision("fp8 scores"):
                            nc.vector.tensor_scalar_add(
                                qt_t[:], ps[:], qb_sb[m][:, 0:1])
                        if SCORES_FP8:
                            if defer_dma is not None:
                                defer_dma.append((qt_relayout, n, m, qt_t))
                            else:
                                qt_relayout(n, m, qt_t)
                        else:
                            qts_all[n][m] = qt_t

                for m in range(HP):
                    for k in range(KE):
                        thunks.append((f, m, k))
                return thunks

            def outproj_thunks(n):
                """32 matmul thunks for the out-projection of s-tile n."""
                state = {}
                thunks = []

                def f(ss, ne, hp):
                    if hp == 0:
                        state[(ss, ne)] = pspool.tile(
                            [128, 512], F32, name="acc_ps", tag="acc", bufs=2)
                        if ne == 0:
                            state[ss] = opool.tile(
                                [128, 1024], F16, name="o_sb", tag="o")
                    ps = state[(ss, ne)]
                    nc.tensor.matmul(
                        ps[:],
                        ots_all[n][hp][:, ss * 128:(ss + 1) * 128],
                        ow_sb[hp][:, ne * 512:(ne + 1) * 512],
                        start=(hp == 0), stop=(hp == HP - 1),
                    )
                    if hp == HP - 1:
                        o_sb = state[ss]
                        nc.vector.tensor_copy(
                            o_sb[:, ne * 512:(ne + 1) * 512], ps[:])
                        # store each half as soon as its copy lands: smaller
                        # final flush, earlier overlap on the serial DMA pipe
                        nc.sync.dma_start(
                            out=out[n * 512 + ss * 128:
                                    n * 512 + (ss + 1) * 128,
                                    ne * 512:(ne + 1) * 512],
                            in_=o_sb[:, ne * 512:(ne + 1) * 512],
                        )

                for ss in range(4):
                    for ne in range(2):
                        for hp in range(HP):
                            thunks.append((f, ss, ne, hp))
                return thunks

            def make_av(n, hp):
                """builders for the attention@V + normalize + transpose
                chain for (n, hp).

                Per s-block sb: 16 matmuls accumulate [s=128, 65]x2 heads into
                one [128,130] psum (col 64 / 129 = softmax denominators from
                the ones column of V), then reciprocal + 2 per-partition
                scalar multiplies normalize into an SBUF tile, and a PE
                transpose (fp16 identity) yields the [d, s] block the
                out-projection consumes.

                Returns (thunks, carry): the normalize/transpose chain lags
                the matmuls by one s-block, and the last block's chain is
                returned as `carry` to be drained at the start of the next
                head pair's steps — the PE transpose sits in the PE stream
                and would otherwise stall it on the DVE norm latency.
                """
                state = {}

                def mm(sb, h2, c):
                    pts = pts_all[(n, hp)]
                    # two s-blocks' [128,130] accumulators packed per psum
                    # bank (regions at col 0 and 256)
                    if c == 0 and h2 == 0 and sb % 2 == 0:
                        state[sb // 2] = pspool.tile(
                            [128, 512], F32, name="ov_ps", tag="ov", bufs=2)
                    ps = state[sb // 2]
                    base = (sb % 2) * 256
                    h = hp * 2 + h2
                    nc.tensor.matmul(
                        ps[:, base + h2 * 65:base + (h2 + 1) * 65],
                        pts[(c // 2, h2)][:, (c % 2) * 512
                                          + sb * 128:(c % 2) * 512
                                          + (sb + 1) * 128],
                        v_sb[c][:, h * 65:(h + 1) * 65],
                        start=(c == 0), stop=(c == CC - 1),
                    )

                def norm(sb):
                    ps = state[sb // 2]
                    base = (sb % 2) * 256
                    rs = spool.tile([128, 2], F32, name="rs", tag="rs")
                    with nc.allow_low_precision("softmax denom"):
                        nc.vector.reciprocal(
                            rs.rearrange("p (g u) -> p g u", u=1),
                            ps[:, base:base + 130]
                            .rearrange("p (g u) -> p g u", u=65)[:, :, 64:65])
                    nt = ntpool.tile([128, 128], F16, name="nt", tag="nt")
                    nc.vector.tensor_scalar_mul(
                        nt[:, 0:64], ps[:, base:base + 64], rs[:, 0:1])
                    nc.vector.tensor_scalar_mul(
                        nt[:, 64:128], ps[:, base + 65:base + 129], rs[:, 1:2])
                    state[(sb, "nt")] = nt

                def transp(sb):
                    # transpose output parks in the unused columns of the
                    # already-allocated ov pair tile (as an fp16 view) — no
                    # extra psum bank, no allocation to wait on
                    nt = state[(sb, "nt")]
                    ps = state[sb // 2]
                    base_tr = (sb % 2) * 256 + 136
                    tr = ps[:, base_tr:base_tr + 64].bitcast(F16)
                    nc.tensor.transpose(tr, nt[:], ident_sb[:])
                    state[(sb, "tr")] = tr

                def trcopy(sb):
                    if sb == 0:
                        ots_all[n][hp] = otpool.tile(
                            [128, 512], CT, name="ot", tag="ot")
                    tr = state[(sb, "tr")]
                    nc.vector.tensor_copy(
                        ots_all[n][hp][:, sb * 128:(sb + 1) * 128], tr)

                def ntc(sb):
                    return [(norm, sb), (transp, sb), (trcopy, sb)]

                return mm, ntc

            def av_thunks(n, hp):
                mm, ntc = make_av(n, hp)

                def mms(sb):
                    return [(mm, sb, h2, c) for h2 in range(2) for c in range(CC)]

                thunks = (mms(0) + mms(1) + ntc(0) + mms(2) + ntc(1)
                          + mms(3) + ntc(2))
                return thunks, ntc(3)

            def run_thunks(ts):
                for f, *args in ts:
                    f(*args)

            # prologue, ordered to match serial DMA arrival (kw+ctx1+kb, qw,
            # x0, vw+ctx2, ow): K^T m=0 first half inline, Q^T(0) m=0 inline;
            # everything else (K^T second half + other head pairs, the whole
            # V projection, remaining Q^T(0) groups) rides in attention(0)'s
            # background, phased by deadline: each head pair's K^T/Q^T lands
            # before the head pair that needs it, V before attention@V(0,0).
            load_x(0)
            defer_dma = []
            run_thunks(kt_thunks(0, t2s=[0]))
            run_thunks(kt_thunks(1, t2s=[0]))   # keeps the PE warm until x0
            qp0 = qproj_thunks(0)
            run_thunks(qp0[:KE])          # m=0 group
            load_late_weights()
            # the deferred fp8 relayouts issue AFTER ctx2/vw so their
            # compute-gated sem waits don't block the late weights in the
            # serial DMA queue
            for fdma, *fargs in defer_dma:
                fdma(*fargs)
            defer_dma = None
            prologue_phases = [
                # kt(0,t2=1) must fully drain before step 5 emits the first
                # c>=4 score matmul (reads emitted before their producer are
                # invisible to the dependency tracker)
                (kt_thunks(0, t2s=[1]) + qp0[KE:2 * KE]
                 + kt_thunks(1, t2s=[1])),
                ([(load_ow,), (vb_chain,)]
                 + [(vproj_group, mc) for mc in range(6)]
                 + kt_thunks(2) + qp0[2 * KE:3 * KE]),
                ([(vproj_group, mc) for mc in range(6, CC)]
                 + kt_thunks(3) + qp0[3 * KE:4 * KE]),
                [],
            ]

            carry = []
            for n in range(NS):
                if n + 1 < NS:
                    load_x(n + 1)
                bg1 = qproj_thunks(n + 1) if n + 1 < NS else []
                bg2 = outproj_thunks(n - 1) if n >= 1 else []

                ots_all[n] = [None] * HP
                qts = qts_all[n]
                n_steps = HP * CC
                step = 0
                bg1_done = 0
                bg2_done = 0
                for hp in range(HP):
                    # attention@V drains lag their head pair by TWO windows
                    # through s-tiles 0/1 (s-tile 0 is PE-oversubscribed by
                    # the projection prologue), catch up to a one-window lag
                    # in s-tile 2's slack, so the epilogue owes only one av
                    i = n * HP + hp
                    if i < 3 * HP:
                        js = [i - 2] if i >= 2 else []
                    elif i == 3 * HP:
                        js = [i - 2, i - 1]
                    else:
                        js = [i - 1]
                    a, newcarry = [], []
                    for j in js:
                        aj, cj = av_thunks(j // HP, j % HP)
                        a += aj
                        newcarry += cj
                    if i == NS * HP - 1:
                        # last window: finish av(3,2) entirely in-window so
                        # the epilogue av's psum slots don't wait on a carried
                        # DVE chain
                        a += newcarry
                        newcarry = []
                    abg = carry + a
                    carry = newcarry
                    abg_done = 0
                    ph = prologue_phases[hp] if n == 0 else []
                    ph_done = 0
                    pts = {}
                    pts_all[(n, hp)] = pts
                    for cp in range(CC // 2):
                      for h2 in range(2):
                        # scores^T for a c-chunk PAIR into one 2-bank psum
                        # tile so a single exp covers 1024 columns (the
                        # per-instruction ACT access overhead is ~30% at 512).
                        # K_h @ Q_h^T, contraction hd=64; h2=0 uses PE rows
                        # 0-63, h2=1 rows 64-127 (row groups).
                        sc = pspool.tile(
                            [128, 1024], F32, name="sc_ps", tag="sc", bufs=2)
                        for ci in range(2):
                            c = 2 * cp + ci
                            if SCORES_FP8:
                                for sh in range(2):
                                    nc.tensor.matmul(
                                        sc[:, ci * 512 + sh * 256:
                                           ci * 512 + (sh + 1) * 256],
                                        kt8v[hp][h2][:, :,
                                                     c * 128:(c + 1) * 128],
                                        qts[hp][h2][:, :,
                                                    sh * 256:(sh + 1) * 256],
                                        start=True, stop=True,
                                        perf_mode=DR,
                                    )
                            else:
                                nc.tensor.matmul(
                                    sc[:, ci * 512:(ci + 1) * 512],
                                    kt_sb[hp][h2 * 64:(h2 + 1) * 64,
                                              c * 128:(c + 1) * 128],
                                    qts[hp][h2 * 64:(h2 + 1) * 64, :],
                                    start=True, stop=True,
                                )
                        p = ptpool.tile([128, 1024], CT, name="pt", tag="pt")
                        nc.scalar.activation(p[:], sc[:], Exp)
                        pts[(cp, h2)] = p
                        # inject background work (attention@V chain for hp-1,
                        # q-proj n+1 / out-proj n-1) between the score steps so
                        # the PE stays busy through the exp latency
                        step += 1
                        stepin = cp * 2 + h2 + 1
                        # startup work (s-tile 0) phased by deadline
                        ptarget = min(len(ph), stepin * len(ph) // (CC - 1))
                        while ph_done < ptarget:
                            f, *args = ph[ph_done]
                            f(*args)
                            ph_done += 1
                        atarget = min(len(abg),
                                      stepin * len(abg) // (CC - 1))
                        while abg_done < atarget:
                            f, *args = abg[abg_done]
                            f(*args)
                            abg_done += 1
                        # q-proj(n+1) over steps 1-16; out-proj(n-1) over
                        # steps 13-32 (its hp=2/3 stationaries are produced
                        # by the lagged av drains in this s-tile's first half)
                        t1 = min(len(bg1), step * len(bg1) // 24)
                        while bg1_done < t1:
                            f, *args = bg1[bg1_done]
                            f(*args)
                            bg1_done += 1
                        t2 = (0 if step <= 12 else
                              min(len(bg2),
                                  (step - 12) * len(bg2) // (n_steps - 12)))
                        while bg2_done < t2:
                            f, *args = bg2[bg2_done]
                            f(*args)
                            bg2_done += 1
                    run_thunks(ph[ph_done:])
                    run_thunks(abg[abg_done:])
                run_thunks(bg1[bg1_done:])
                run_thunks(bg2[bg2_done:])

            # epilogue: attention@V for the last head pair interleaved with
            # the out-projection of the last s-tile.
            # av layout: [carry][mms0 16][mms1 16][ntc0 3][mms2 16][ntc1 3]
            # [mms3 16][ntc2 3][ntc3 3]; out-proj group ss needs ntc(ss).
            a, newcarry = av_thunks(NS - 1, HP - 1)
            av_all = carry + a + newcarry
            oth = outproj_thunks(NS - 1)
            nca = len(carry)
            cuts = [nca + 35, nca + 54, nca + 73, len(av_all)]
            merged = []
            ai = 0
            for ss in range(4):
                merged += av_all[ai:cuts[ss]]
                merged += oth[ss * 8:(ss + 1) * 8]
                ai = cuts[ss]
            run_thunks(merged)

          # timing aid: calibrated delay chain on the otherwise-idle gpsimd
          # engine; kernel exec time = max(real work, nop chain)
          if nop_us:
            NOP_CYC = 48000  # 40 us at 1.2 GHz
            for _ in range(int(nop_us * 1200 / NOP_CYC)):
                nc.gpsimd.nop(cycle_cnt=NOP_CYC, nofuse=True)

    nc.finalize()
    return nc


def kernel(x, context, q_w, q_b, k_w, k_b, v_w, v_b, o_w, o_b):
    global _built, _last_results
    from concourse.bass_utils import run_bass_kernel_spmd

    if _built is None:
        _built = _build()
    nc = _built

    scale = np.float32(1.0 / np.sqrt(HD))
    ind = np.float32 if DTYPE_MODE == "fp32r" else np.float16
    x = np.asarray(x, np.float32)
    context = np.asarray(context, np.float32)
    xTs = [np.ascontiguousarray(x[b].T).astype(ind) for b in range(B)]
    ctxTs = [np.ascontiguousarray(context[b].T).astype(ind) for b in range(B)]

    in_maps = []
    for core in range(N_CORES):
        b, hg = core // 2, core % 2
        el = slice(hg * EL, (hg + 1) * EL)
        in_maps.append({
            "xT": xTs[b],
            "ctxT": ctxTs[b],
            "qw": np.ascontiguousarray(
                (np.asarray(q_w, np.float32)[:, el] * scale).astype(ind)),
            "kw": np.ascontiguousarray(np.asarray(k_w, np.float32)[:, el]).astype(ind),
            "vw": np.ascontiguousarray(np.asarray(v_w, np.float32)[:, el]).astype(ind),
            "ow": np.ascontiguousarray(np.asarray(o_w, np.float32)[el, :]).astype(ind),
            "qb": np.ascontiguousarray(
                (np.asarray(q_b, np.float32)[el] * scale)[:, None]),
            "kb": np.ascontiguousarray(np.asarray(k_b, np.float32)[el][:, None]),
            "vb": np.ascontiguousarray(
                np.asarray(v_b, np.float32)[el][None, :]).astype(ind),
            "ones_r": np.ones((1, 128), ind),
            "ident": np.eye(128, dtype=np.float16),
        })

    res = run_bass_kernel_spmd(nc, in_maps, list(range(N_CORES)))
    _last_results = res

    ob = np.asarray(o_b, np.float32)
    full = np.empty((B, S, E), np.float32)
    for b in range(B):
        full[b] = (res.results[2 * b]["out"].astype(np.float32)
                   + res.results[2 * b + 1]["out"].astype(np.float32) + ob)
    return full
